# revision 33
# baseline (speedup 1.0000x reference)
"""DiversityAttention on 8 TRN2 NeuronCores (Bass/Tile).

Sharding: data-parallel over batch (B=2) x tensor-parallel over heads
(16 heads -> 4 groups of 4). core = (b, g), b = core // 4, g = core % 4.
Each core computes full attention for its 4 heads over its batch and a
partial out-projection [S, HIDDEN]; the host sums the 4 partials per
batch and adds bo.

All big matmuls run as fp8e4 DoubleRow (0.25x the fp32r cost) where
precision allows; q/k/v projections use a 3-term fp8 residual
decomposition (W8@x8 + W8@xr8 + Wr8@x8 ~ W@x to ~0.1%).

Scale conventions (host-side):
  wq8/wqr8 = fp8(256 * Wq^T / sqrt(dh)), bq' = 256*bq/sqrt(dh)
  wk8/wkr8 = fp8(256 * Wk^T),            bk' = 256*bk
  wv8/wvr8 = fp8(256 * Wv^T),            bv' = 256*bv
  x8/xr8   = fp8(x^T) + fp8 residual
  xh8      = fp8(sqrt(gamma) * 256 * x^T / max(||x||, eps))
so on device:
  qT_sb = 256*q, kT_sb = 256*k   -> scores_psum = 65536 * s
  sim_psum = 65536 * gamma * sim -> s_t = 65536*(s - gamma*sim)
  P = exp(s_t / 65536)  (activation scale)
  v2 = [256 | 256*v] per head (ones col at 0 for the denominator row);
  ctx_psum row 0 = 256*sum(P), rows 1..64 = 256*sum(v P): ratio exact.
Division runs fully on GpSimd: partition_broadcast(denom row 0) ->
reciprocal -> multiply, keeping the DVE queue free for the score-sim
subtractions.
"""

import math
import os
import sys

import numpy as np

for _p in ("/opt/trn_rl_repo",):
    if _p not in sys.path and os.path.isdir(_p):
        sys.path.insert(0, _p)

os.environ.setdefault("MYCRO_LOCAL_CACHE", "1")

import ml_dtypes

import concourse.bass as bass
import concourse.tile as tile
from concourse import bacc, mybir
from concourse.bass_utils import run_bass_kernel_spmd


def _install_ntff_hook():
    """Provide antenv.axon_hooks (NTFF profiling registry) if the image
    lacks it, mirroring trn_agent_boot's ctypes hook. No-op on failure."""
    try:
        import antenv.axon_hooks  # noqa: F401
        return
    except ImportError:
        pass
    try:
        import contextlib
        import ctypes
        import types

        so_path = "/opt/axon/libaxon_pjrt.so"
        if not os.path.exists(so_path):
            return
        lib = ctypes.CDLL(so_path)
        if not hasattr(lib, "axon_start_nrt_profile"):
            return
        lib.axon_start_nrt_profile.argtypes = [
            ctypes.POINTER(ctypes.c_int64), ctypes.c_size_t]
        lib.axon_start_nrt_profile.restype = ctypes.c_int64
        lib.axon_stop_nrt_profile.argtypes = [ctypes.c_char_p]
        lib.axon_stop_nrt_profile.restype = ctypes.c_int64

        @contextlib.contextmanager
        def _hook(output_dir, device_ids):
            import jax
            jax.devices()
            if device_ids:
                ids = (ctypes.c_int64 * len(device_ids))(*device_ids)
                rc = lib.axon_start_nrt_profile(ids, len(device_ids))
            else:
                rc = lib.axon_start_nrt_profile(None, 0)
            if rc != 0:
                raise RuntimeError(f"axon_start_nrt_profile rc={rc}")
            try:
                yield
            finally:
                n = lib.axon_stop_nrt_profile(str(output_dir).encode())
                print(f"ntff profile: {n} file(s) -> {output_dir}",
                      file=sys.stderr)

        mod = types.ModuleType("antenv.axon_hooks")
        _state = {"hook": _hook}
        mod.set_axon_ntff_profile_hook = lambda h: _state.__setitem__("hook", h)
        mod.get_axon_ntff_profile_hook = lambda: _state["hook"]
        sys.modules["antenv.axon_hooks"] = mod
        import antenv
        antenv.axon_hooks = mod
    except Exception:
        pass


_install_ntff_hook()

F32 = mybir.dt.float32
F32R = mybir.dt.float32r
BF16 = mybir.dt.bfloat16
FP8 = mybir.dt.float8e4
NP_FP8 = ml_dtypes.float8_e4m3
ACT_EXP = mybir.ActivationFunctionType.Exp
ACT_COPY = mybir.ActivationFunctionType.Copy
DOUBLE_ROW = mybir.MatmulPerfMode.DoubleRow

# Problem constants (hardcoded per contract).
HIDDEN = 1024
HEADS = 16
HEAD_DIM = 64
GAMMA = 0.5
B, S = 2, 2048
N_CORES = 8
GROUPS = N_CORES // B  # head groups per batch
HPC = HEADS // GROUPS  # heads per core
LAG = 3  # kt software-pipeline lag between exp and ctx matmul
WSCALE = 256.0
EXP_DESCALE = 1.0 / (WSCALE * WSCALE)


def emit_kernel(tc, aps, *, S_, C_, HPC_, QB):
    """Emit the per-core kernel. aps: dict of dram APs."""
    nc = tc.nc
    CT = C_ // 128          # contraction tiles over hidden
    CP = CT // 2            # contraction pairs for fp8 DoubleRow
    PAIRS = HPC_ // 2       # head pairs (128-channel chunks)
    D2 = HPC_ * HEAD_DIM
    NKT = S_ // 128         # key tiles
    NQB = S_ // QB          # query blocks
    PB = min(512, S_)       # projection free-block width
    NPB = S_ // PB
    KPB = PB // 128         # key tiles per nb block
    OB_W = min(512, C_)     # out-projection free-block width
    NOB = C_ // OB_W

    xh8_d = aps["xh8"]; xT_d = aps["xT"]
    wq_d = aps["wq"]; wk_d = aps["wk"]; wv_d = aps["wv"]; wo_d = aps["wo"]
    bq_d = aps["bq"]; bk_d = aps["bk"]; bv_d = aps["bv"]
    out_d = aps["out"]
    mask_d = aps.get("maskadd")

    from contextlib import ExitStack
    stack = ExitStack()
    consts = stack.enter_context(tc.tile_pool(name="consts", bufs=1))
    xpool = stack.enter_context(tc.tile_pool(name="xpool", bufs=1))
    projpool = stack.enter_context(tc.tile_pool(name="projpool", bufs=1))

    wo_sb = consts.tile([128, PAIRS, C_], F32R)
    xh8_sb = xpool.tile([128, CT, S_], FP8)

    # projections (fp32r, pre-scaled by 256; V in [keys, 1+dims] layout
    # with the 256-valued denominator column at position 0)
    qT_sb = projpool.tile([128, PAIRS, S_], F32R)
    kT_sb = projpool.tile([128, PAIRS, S_], F32R)
    v2_sb = projpool.tile([128, HPC_, NKT, HEAD_DIM + 1], F32R)

    with tc.tile_pool(name="xtpool", bufs=1) as xtpool, \
         tc.tile_pool(name="wstage", bufs=1) as wstage, \
         tc.tile_pool(name="wpool", bufs=1) as wpool, \
         tc.tile_pool(name="ph1psum", bufs=2, space="PSUM") as prj_ps, \
         tc.tile_pool(name="vpsum", bufs=2, space="PSUM") as v_ps:
        # x^T arrives as bf16 (host cast); per-nb tiles so the first
        # projection block starts after ~1MB of DMA instead of 4MB
        xTnb = [xtpool.tile([128, CT, PB], BF16, name=f"xT_{nb}")
                for nb in range(NPB)]
        wq_sb = wpool.tile([128, CT, D2], BF16)
        wk_sb = wpool.tile([128, CT, D2], BF16)
        wv_sb = wpool.tile([128, CT, D2], BF16)
        ws = wstage.tile([128, CT, D2], F32, tag="ws", name="wqs")
        nc.sync.dma_start(out=ws, in_=wq_d.rearrange("(t p) m -> p t m", p=128))
        nc.vector.tensor_copy(wq_sb, ws)
        xT_r = xT_d.rearrange("(t p) m -> p t m", p=128)
        for nb in range(NPB):
            nc.sync.dma_start(out=xTnb[nb],
                              in_=xT_r[:, :, nb * PB:(nb + 1) * PB])
        for w_sb, w_d, wn in ((wk_sb, wk_d, "wks"), (wv_sb, wv_d, "wvs")):
            ws = wstage.tile([128, CT, D2], F32, tag="ws", name=wn)
            nc.sync.dma_start(out=ws, in_=w_d.rearrange("(t p) m -> p t m", p=128))
            nc.vector.tensor_copy(w_sb, ws)
        bq_sb = wpool.tile([128, PAIRS, 1], F32)
        bk_sb = wpool.tile([128, PAIRS, 1], F32)
        for b_sb, b_d in ((bq_sb, bq_d), (bk_sb, bk_d)):
            nc.sync.dma_start(
                out=b_sb, in_=b_d.rearrange("(j p) one -> p j one", p=128))
        vbias_sb = wpool.tile([128, D2], F32)
        nc.sync.dma_start(out=vbias_sb, in_=bv_d.to_broadcast([128, D2]))
        for c in range(CT):
            nc.sync.dma_start(out=xh8_sb[:, c, :],
                              in_=xh8_d[c * 128:(c + 1) * 128, :])
        wos = wstage.tile([128, PAIRS, C_], F32, tag="ws", name="wos")
        nc.sync.dma_start(out=wos, in_=wo_d.rearrange("(j p) o -> p j o", p=128))
        nc.vector.tensor_copy(wo_sb, wos)

        for w_sb, b_sb, dest in (
            (wq_sb, bq_sb, qT_sb),
            (wk_sb, bk_sb, kT_sb),
        ):
            for nb in range(NPB):
                pss = [prj_ps.tile([128, PB], F32, tag=f"prj{j}",
                                   name=f"prj_{dest.tensor.name}_{nb}_{j}")
                       for j in range(PAIRS)]
                for c in range(CT):
                    for j in range(PAIRS):
                        nc.tensor.matmul(
                            pss[j],
                            w_sb[:, c, j * 128:(j + 1) * 128],
                            xTnb[nb][:, c, :],
                            start=(c == 0),
                            stop=(c == CT - 1),
                        )
                for j in range(PAIRS):
                    nc.vector.tensor_scalar_add(
                        dest[:, j, nb * PB:(nb + 1) * PB], pss[j], b_sb[:, j, :]
                    )

        # V computed directly with keys on partitions (lhsT = x^T chunk)
        for kt in range(NKT):
            nb, off = divmod(kt, PB // 128)
            off *= 128
            vp = v_ps.tile([128, D2], F32, tag="vp")
            for c in range(CT):
                nc.tensor.matmul(
                    vp,
                    xTnb[nb][:, c, off:off + 128],
                    wv_sb[:, c, :],
                    start=(c == 0),
                    stop=(c == CT - 1),
                )
            nc.vector.tensor_add(
                v2_sb[:, :, kt, 0:HEAD_DIM],
                vp.rearrange("p (h d) -> p h d", h=HPC_),
                vbias_sb.rearrange("p (h d) -> p h d", h=HPC_),
            )
        onescol = wstage.tile([128, HPC_, NKT, 1], F32)
        nc.vector.memset(onescol, WSCALE)
        nc.vector.tensor_copy(v2_sb[:, :, :, HEAD_DIM:HEAD_DIM + 1], onescol)

    # per-qb ctx tiles so the out-projection of early query blocks does not
    # serialize on the last block's divisions (whole-tile dependency)
    ctxT2_nq = [projpool.tile([128, PAIRS, QB], F32R, name=f"ctxT2_{nq}")
                for nq in range(NQB)]

    # --- main loop (phase 2) ---
    ptpool = stack.enter_context(tc.tile_pool(name="ptpool", bufs=9))
    spool = stack.enter_context(tc.tile_pool(name="spool", bufs=2))
    simsb = stack.enter_context(tc.tile_pool(name="simsb", bufs=2))
    smallpool = stack.enter_context(tc.tile_pool(name="smallpool", bufs=2))
    mpool = (stack.enter_context(tc.tile_pool(name="mpool", bufs=2))
             if mask_d is not None else None)

    with tc.tile_pool(name="simpsum", bufs=2, space="PSUM") as simp, \
         tc.tile_pool(name="scpsum", bufs=1, space="PSUM") as scp, \
         tc.tile_pool(name="ctxpsum", bufs=1, space="PSUM") as ctxp:

        def emit_ctx(ctx_ps, kt, pt_pairs, js=None):
            for j in range(PAIRS) if js is None else js:
                for hi in range(2):
                    nc.tensor.matmul(
                        ctx_ps[2 * j + hi],
                        v2_sb[:, 2 * j + hi, kt, :],
                        pt_pairs[j][:, hi, :],
                        start=(kt == 0),
                        stop=(kt == NKT - 1),
                        skip_group_check=True,
                    )

        def emit_division_head(qb, ctx_ps, h):
            # short chain: single-op approx reciprocal of the denominator
            # row (DVE, PSUM read), GpSimd partition-0 broadcast, DVE mul.
            j, hi = divmod(h, 2)
            r0c = smallpool.tile([1, QB], F32, tag=f"r0c{h % 2}",
                                 name=f"r0c_{qb}_{h}")
            nc.vector.tensor_copy(r0c, ctx_ps[h][HEAD_DIM:HEAD_DIM + 1, :])
            r0 = smallpool.tile([1, QB], F32, tag=f"r0{h % 2}",
                                name=f"r0_{qb}_{h}")
            nc.vector.reciprocal_approx_fast(out=r0, in_=r0c)
            rb = smallpool.tile([HEAD_DIM, QB], F32, tag="rb",
                                name=f"rb_{qb}_{h}")
            nc.gpsimd.partition_broadcast(rb, r0, channels=HEAD_DIM)
            nc.vector.tensor_mul(
                ctxT2_nq[qb][hi * 64:hi * 64 + 64, j, :],
                ctx_ps[h][0:HEAD_DIM, :],
                rb,
            )

        def emit_division(qb, ctx_ps):
            for h in range(HPC_):
                emit_division_head(qb, ctx_ps, h)

        prev_div = None
        for qb in range(NQB):
            ctx_ps = [ctxp.tile([HEAD_DIM + 1, QB], F32, tag=f"ctx{h}",
                                name=f"ctx_{qb}_{h}")
                      for h in range(HPC_)]
            pending = []
            ctx_tail = None
            for kt in range(NKT):
                sp = simp.tile([128, QB], F32, tag="sim")
                for cp in range(CP):
                    nc.tensor.matmul(
                        sp,
                        xh8_sb[:, 2 * cp:2 * cp + 2, kt * 128:(kt + 1) * 128],
                        xh8_sb[:, 2 * cp:2 * cp + 2, qb * QB:(qb + 1) * QB],
                        start=(cp == 0),
                        stop=(cp == CP - 1),
                        perf_mode=DOUBLE_ROW,
                    )
                # HW allows only one PSUM operand per DVE op: stage sim in
                # SBUF (alternate ACT/DVE to balance engine load)
                sim_t = simsb.tile([128, QB], F32, tag="simsb")
                if kt % 2 == 0:
                    nc.scalar.activation(out=sim_t, in_=sp, func=ACT_COPY)
                else:
                    nc.vector.tensor_copy(sim_t, sp)
                sim_in = sim_t.unsqueeze(1).to_broadcast([128, 2, QB])
                if mask_d is not None:
                    m_sb = mpool.tile([128, QB], F32, tag="msk")
                    nc.sync.dma_start(
                        out=m_sb,
                        in_=mask_d[kt * 128:(kt + 1) * 128, qb * QB:(qb + 1) * QB],
                    )
                pt_pairs = []
                for j in range(PAIRS):
                    sc_t = scp.tile([128, 2, QB], F32, tag="scp")
                    for hi in range(2):
                        pr = slice(hi * 64, hi * 64 + 64)
                        nc.tensor.matmul(
                            sc_t[:, hi, :],
                            kT_sb[pr, j, kt * 128:(kt + 1) * 128],
                            qT_sb[pr, j, qb * QB:(qb + 1) * QB],
                            start=True,
                            stop=True,
                        )
                    # interleave lagged ctx matmuls: half between the two
                    # score pairs, half after them — so the PE has filler
                    # both while the DVE runs sub(j0) and, at the iteration
                    # boundary, while it finishes sub(j1)
                    if j == 0 and pending and len(pending) > LAG:
                        k0, p0 = pending.pop(0)
                        emit_ctx(ctx_ps, k0, p0, js=(0,))
                        ctx_tail = (k0, p0)
                    s_t = spool.tile([128, 2, QB], F32, tag="s")
                    nc.vector.tensor_sub(s_t, sc_t, sim_in)
                    if mask_d is not None:
                        nc.vector.tensor_sub(
                            s_t, s_t, m_sb.unsqueeze(1).to_broadcast([128, 2, QB]))
                    pt = ptpool.tile([128, 2, QB], F32R, tag="pt")
                    nc.scalar.activation(out=pt, in_=s_t, func=ACT_EXP,
                                         scale=EXP_DESCALE)
                    pt_pairs.append(pt)
                if ctx_tail is not None:
                    emit_ctx(ctx_ps, ctx_tail[0], ctx_tail[1], js=(1,))
                    ctx_tail = None
                pending.append((kt, pt_pairs))
                # previous block's divisions, spread over the kt loop on GpSimd
                if prev_div is not None and kt >= 2 and (kt - 2) % 3 == 0:
                    h = (kt - 2) // 3
                    if h < HPC_:
                        emit_division_head(prev_div[0], prev_div[1], h)
                        if h == HPC_ - 1:
                            prev_div = None
            for k0, p0 in pending:
                emit_ctx(ctx_ps, k0, p0)
            if prev_div is not None:
                done = max(0, (NKT - 1 - 2) // 3 + 1) if NKT > 2 else 0
                for h in range(min(done, HPC_), HPC_):
                    emit_division_head(prev_div[0], prev_div[1], h)
                prev_div = None
            prev_div = (qb, ctx_ps)
        emit_division(*prev_div)

    # --- out-projection (phase 3) ---
    with tc.tile_pool(name="outpsum", bufs=4, space="PSUM") as outp, \
         tc.tile_pool(name="outstg", bufs=4) as outstg:
        QT_PER = QB // 128
        for qt in range(S_ // 128):
            for ob in range(NOB):
                op = outp.tile([128, OB_W], F32, tag="op")
                for j in range(PAIRS):
                    nc.tensor.matmul(
                        op,
                        ctxT2_nq[qt // QT_PER][
                            :, j, (qt % QT_PER) * 128:(qt % QT_PER + 1) * 128],
                        wo_sb[:, j, ob * OB_W:(ob + 1) * OB_W],
                        start=(j == 0),
                        stop=(j == PAIRS - 1),
                    )
                ostg = outstg.tile([128, OB_W], F32, tag="ostg")
                if (qt + ob) % 2 == 0:
                    nc.scalar.activation(out=ostg, in_=op, func=ACT_COPY)
                else:
                    nc.vector.tensor_copy(ostg, op)
                nc.sync.dma_start(
                    out=out_d[qt * 128:(qt + 1) * 128, ob * OB_W:(ob + 1) * OB_W],
                    in_=ostg,
                )

    stack.close()


def build_nc(*, S_=S, C_=HIDDEN, HPC_=HPC, QB=512, with_mask=False,
             enable_asserts=False):
    nc = bacc.Bacc(
        "TRN2", target_bir_lowering=False, debug=False,
        enable_asserts=enable_asserts,
    )
    D2 = HPC_ * HEAD_DIM
    aps = {}
    aps["xT"] = nc.dram_tensor("xT", [C_, S_], BF16, kind="ExternalInput").ap()
    aps["xh8"] = nc.dram_tensor("xh8", [C_, S_], FP8, kind="ExternalInput").ap()
    for n in ("wq", "wk", "wv"):
        aps[n] = nc.dram_tensor(n, [C_, D2], F32, kind="ExternalInput").ap()
    aps["wo"] = nc.dram_tensor("wo", [D2, C_], F32, kind="ExternalInput").ap()
    for n in ("bq", "bk"):
        aps[n] = nc.dram_tensor(n, [D2, 1], F32, kind="ExternalInput").ap()
    aps["bv"] = nc.dram_tensor("bv", [1, D2], F32, kind="ExternalInput").ap()
    if with_mask:
        aps["maskadd"] = nc.dram_tensor(
            "maskadd", [S_, S_], F32, kind="ExternalInput").ap()
    aps["out"] = nc.dram_tensor("out", [S_, C_], F32, kind="ExternalOutput").ap()

    with tile.TileContext(nc) as tc:
        emit_kernel(tc, aps, S_=S_, C_=C_, HPC_=HPC_, QB=QB)
    nc.compile()
    return nc


def _q8(a):
    return a.astype(NP_FP8)


def host_prepare(x, attn_mask, Wq, bq, Wk, bk, Wv, bv, Wo, bo, *,
                 S_=S, C_=HIDDEN, HPC_=HPC, n_cores=N_CORES):
    """Build the per-core input maps. Returns (in_maps, with_mask)."""
    x = np.asarray(x, np.float32)
    B_ = x.shape[0]
    groups = n_cores // B_
    Wq = np.asarray(Wq, np.float32); Wk = np.asarray(Wk, np.float32)
    Wv = np.asarray(Wv, np.float32); Wo = np.asarray(Wo, np.float32)
    bq = np.asarray(bq, np.float32); bk = np.asarray(bk, np.float32)
    bv = np.asarray(bv, np.float32)

    inv_sqrt_d = 1.0 / math.sqrt(HEAD_DIM)
    # weights carry the 256x matched scale so scores_psum = 65536 * s
    WqT = np.ascontiguousarray(Wq.T * (inv_sqrt_d * WSCALE)).astype(np.float32)
    WkT = np.ascontiguousarray(Wk.T * WSCALE).astype(np.float32)
    WvT = np.ascontiguousarray(Wv.T * WSCALE).astype(np.float32)
    WoT = np.ascontiguousarray(Wo.T)                 # [C(c), C(o)]
    bq_s = bq * (inv_sqrt_d * WSCALE)
    bk_s = bk * WSCALE
    bv_s = bv * WSCALE

    mask = np.asarray(attn_mask)
    with_mask = bool(mask.any())
    maskadd = None
    if with_mask:
        # s_t -= maskadd; masked positions get -1e20/65536 pre-exp -> 0
        maskadd = np.where(mask, np.float32(1e20), np.float32(0.0)).astype(np.float32)
        maskadd = np.ascontiguousarray(maskadd.T)  # [k, q]

    in_maps = []
    per_b = {}
    for core in range(n_cores):
        b, g = divmod(core, groups)
        if b not in per_b:
            xb = x[b]                               # [S, C]
            xT = np.ascontiguousarray(xb.T)         # [C, S]
            norms = np.linalg.norm(xb, axis=1)      # [S]
            scale = (math.sqrt(GAMMA) * WSCALE /
                     np.maximum(norms, 1e-12)).astype(np.float32)
            xh8 = _q8(xT * scale[None, :])
            xT16 = np.ascontiguousarray(xT.astype(ml_dtypes.bfloat16))
            per_b[b] = (xT16, np.ascontiguousarray(xh8))
        xT, xh8 = per_b[b]
        ch = slice(g * HPC_ * HEAD_DIM, (g + 1) * HPC_ * HEAD_DIM)
        m = {
            "xT": xT, "xh8": xh8,
            "wq": np.ascontiguousarray(WqT[:, ch]),
            "wk": np.ascontiguousarray(WkT[:, ch]),
            "wv": np.ascontiguousarray(WvT[:, ch]),
            "wo": np.ascontiguousarray(WoT[ch, :]),
            "bq": np.ascontiguousarray(bq_s[ch]).reshape(-1, 1),
            "bk": np.ascontiguousarray(bk_s[ch]).reshape(-1, 1),
            "bv": np.ascontiguousarray(bv_s[ch]).reshape(1, -1),
        }
        if with_mask:
            m["maskadd"] = maskadd
        in_maps.append(m)
    return in_maps, with_mask


_NC_CACHE = {}


def _get_nc(with_mask):
    key = with_mask
    if key not in _NC_CACHE:
        _NC_CACHE[key] = build_nc(with_mask=with_mask)
    return _NC_CACHE[key]


LAST_RESULTS = None


def kernel(**inputs):
    global LAST_RESULTS
    in_maps, with_mask = host_prepare(
        inputs["x"], inputs["attn_mask"],
        inputs["Wq"], inputs["bq"], inputs["Wk"], inputs["bk"],
        inputs["Wv"], inputs["bv"], inputs["Wo"], inputs["bo"],
    )
    nc = _get_nc(with_mask)
    res = run_bass_kernel_spmd(nc, in_maps, core_ids=list(range(N_CORES)))
    LAST_RESULTS = res
    bo = np.asarray(inputs["bo"], np.float32)
    out = np.zeros((B, S, HIDDEN), np.float32)
    groups = N_CORES // B
    for core in range(N_CORES):
        b = core // groups
        out[b] += res.results[core]["out"]
    out += bo[None, None, :]
    return out


# revision 47
# speedup vs baseline: 1.0009x; 1.0009x over previous
"""DiversityAttention on 8 TRN2 NeuronCores (Bass/Tile).

Sharding: data-parallel over batch (B=2) x tensor-parallel over heads
(16 heads -> 4 groups of 4). core = (b, g), b = core // 4, g = core % 4.
Each core computes full attention for its 4 heads over its batch and a
partial out-projection [S, HIDDEN]; the host sums the 4 partials per
batch and adds bo.

All big matmuls run as fp8e4 DoubleRow (0.25x the fp32r cost) where
precision allows; q/k/v projections use a 3-term fp8 residual
decomposition (W8@x8 + W8@xr8 + Wr8@x8 ~ W@x to ~0.1%).

Scale conventions (host-side):
  wq8/wqr8 = fp8(256 * Wq^T / sqrt(dh)), bq' = 256*bq/sqrt(dh)
  wk8/wkr8 = fp8(256 * Wk^T),            bk' = 256*bk
  wv8/wvr8 = fp8(256 * Wv^T),            bv' = 256*bv
  x8/xr8   = fp8(x^T) + fp8 residual
  xh8      = fp8(sqrt(gamma) * 256 * x^T / max(||x||, eps))
so on device:
  qT_sb = 256*q, kT_sb = 256*k   -> scores_psum = 65536 * s
  sim_psum = 65536 * gamma * sim -> s_t = 65536*(s - gamma*sim)
  P = exp(s_t / 65536)  (activation scale)
  v2 = [256 | 256*v] per head (ones col at 0 for the denominator row);
  ctx_psum row 0 = 256*sum(P), rows 1..64 = 256*sum(v P): ratio exact.
Division runs fully on GpSimd: partition_broadcast(denom row 0) ->
reciprocal -> multiply, keeping the DVE queue free for the score-sim
subtractions.
"""

import math
import os
import sys

import numpy as np

for _p in ("/opt/trn_rl_repo",):
    if _p not in sys.path and os.path.isdir(_p):
        sys.path.insert(0, _p)

os.environ.setdefault("MYCRO_LOCAL_CACHE", "1")

import ml_dtypes

import concourse.bass as bass
import concourse.tile as tile
from concourse import bacc, mybir
from concourse.bass_utils import run_bass_kernel_spmd


def _install_ntff_hook():
    """Provide antenv.axon_hooks (NTFF profiling registry) if the image
    lacks it, mirroring trn_agent_boot's ctypes hook. No-op on failure."""
    try:
        import antenv.axon_hooks  # noqa: F401
        return
    except ImportError:
        pass
    try:
        import contextlib
        import ctypes
        import types

        so_path = "/opt/axon/libaxon_pjrt.so"
        if not os.path.exists(so_path):
            return
        lib = ctypes.CDLL(so_path)
        if not hasattr(lib, "axon_start_nrt_profile"):
            return
        lib.axon_start_nrt_profile.argtypes = [
            ctypes.POINTER(ctypes.c_int64), ctypes.c_size_t]
        lib.axon_start_nrt_profile.restype = ctypes.c_int64
        lib.axon_stop_nrt_profile.argtypes = [ctypes.c_char_p]
        lib.axon_stop_nrt_profile.restype = ctypes.c_int64

        @contextlib.contextmanager
        def _hook(output_dir, device_ids):
            import jax
            jax.devices()
            if device_ids:
                ids = (ctypes.c_int64 * len(device_ids))(*device_ids)
                rc = lib.axon_start_nrt_profile(ids, len(device_ids))
            else:
                rc = lib.axon_start_nrt_profile(None, 0)
            if rc != 0:
                raise RuntimeError(f"axon_start_nrt_profile rc={rc}")
            try:
                yield
            finally:
                n = lib.axon_stop_nrt_profile(str(output_dir).encode())
                print(f"ntff profile: {n} file(s) -> {output_dir}",
                      file=sys.stderr)

        mod = types.ModuleType("antenv.axon_hooks")
        _state = {"hook": _hook}
        mod.set_axon_ntff_profile_hook = lambda h: _state.__setitem__("hook", h)
        mod.get_axon_ntff_profile_hook = lambda: _state["hook"]
        sys.modules["antenv.axon_hooks"] = mod
        import antenv
        antenv.axon_hooks = mod
    except Exception:
        pass


_install_ntff_hook()

F32 = mybir.dt.float32
F32R = mybir.dt.float32r
BF16 = mybir.dt.bfloat16
FP8 = mybir.dt.float8e4
NP_FP8 = ml_dtypes.float8_e4m3
ACT_EXP = mybir.ActivationFunctionType.Exp
ACT_COPY = mybir.ActivationFunctionType.Copy
DOUBLE_ROW = mybir.MatmulPerfMode.DoubleRow

# Problem constants (hardcoded per contract).
HIDDEN = 1024
HEADS = 16
HEAD_DIM = 64
GAMMA = 0.5
B, S = 2, 2048
N_CORES = 8
GROUPS = N_CORES // B  # head groups per batch
HPC = HEADS // GROUPS  # heads per core
LAG = 3  # kt software-pipeline lag between exp and ctx matmul
WSCALE = 256.0
EXP_DESCALE = 1.0 / (WSCALE * WSCALE)


def emit_kernel(tc, aps, *, S_, C_, HPC_, QB):
    """Emit the per-core kernel. aps: dict of dram APs."""
    nc = tc.nc
    CT = C_ // 128          # contraction tiles over hidden
    CP = CT // 2            # contraction pairs for fp8 DoubleRow
    PAIRS = HPC_ // 2       # head pairs (128-channel chunks)
    D2 = HPC_ * HEAD_DIM
    NKT = S_ // 128         # key tiles
    NQB = S_ // QB          # query blocks
    PB = min(512, S_)       # projection free-block width
    NPB = S_ // PB
    KPB = PB // 128         # key tiles per nb block
    OB_W = min(512, C_)     # out-projection free-block width
    NOB = C_ // OB_W

    xh8_d = aps["xh8"]; xT_d = aps["xT"]
    wq_d = aps["wq"]; wk_d = aps["wk"]; wv_d = aps["wv"]; wo_d = aps["wo"]
    bq_d = aps["bq"]; bk_d = aps["bk"]; bv_d = aps["bv"]
    out_d = aps["out"]
    mask_d = aps.get("maskadd")

    from contextlib import ExitStack
    stack = ExitStack()
    consts = stack.enter_context(tc.tile_pool(name="consts", bufs=1))
    xpool = stack.enter_context(tc.tile_pool(name="xpool", bufs=1))
    projpool = stack.enter_context(tc.tile_pool(name="projpool", bufs=1))

    wo_sb = consts.tile([128, PAIRS, C_], F32R)
    xh8_sb = xpool.tile([128, CT, S_], FP8)

    # projections (fp32r, pre-scaled by 256; V in [keys, 1+dims] layout
    # with the 256-valued denominator column at position 0)
    qT_sb = projpool.tile([128, PAIRS, S_], F32R)
    kT_sb = projpool.tile([128, PAIRS, S_], F32R)
    v2_sb = projpool.tile([128, HPC_, NKT, HEAD_DIM + 1], F32R)

    with tc.tile_pool(name="xtpool", bufs=1) as xtpool, \
         tc.tile_pool(name="wstage", bufs=1) as wstage, \
         tc.tile_pool(name="wpool", bufs=1) as wpool, \
         tc.tile_pool(name="ph1psum", bufs=2, space="PSUM") as prj_ps, \
         tc.tile_pool(name="vpsum", bufs=2, space="PSUM") as v_ps:
        # x^T arrives as bf16 (host cast); per-nb tiles so the first
        # projection block starts after ~1MB of DMA instead of 4MB
        xTnb = [xtpool.tile([128, CT, PB], BF16, name=f"xT_{nb}")
                for nb in range(NPB)]
        wq_sb = wpool.tile([128, CT, D2], BF16)
        wk_sb = wpool.tile([128, CT, D2], BF16)
        wv_sb = wpool.tile([128, CT, D2], BF16)
        ws = wstage.tile([128, CT, D2], F32, tag="ws", name="wqs")
        nc.sync.dma_start(out=ws, in_=wq_d.rearrange("(t p) m -> p t m", p=128))
        nc.vector.tensor_copy(wq_sb, ws)
        xT_r = xT_d.rearrange("(t p) m -> p t m", p=128)
        for nb in range(NPB):
            nc.sync.dma_start(out=xTnb[nb],
                              in_=xT_r[:, :, nb * PB:(nb + 1) * PB])
        for w_sb, w_d, wn in ((wk_sb, wk_d, "wks"), (wv_sb, wv_d, "wvs")):
            ws = wstage.tile([128, CT, D2], F32, tag="ws", name=wn)
            nc.sync.dma_start(out=ws, in_=w_d.rearrange("(t p) m -> p t m", p=128))
            nc.vector.tensor_copy(w_sb, ws)
        bq_sb = wpool.tile([128, PAIRS, 1], F32)
        bk_sb = wpool.tile([128, PAIRS, 1], F32)
        for b_sb, b_d in ((bq_sb, bq_d), (bk_sb, bk_d)):
            nc.sync.dma_start(
                out=b_sb, in_=b_d.rearrange("(j p) one -> p j one", p=128))
        vbias_sb = wpool.tile([128, D2], F32)
        nc.sync.dma_start(out=vbias_sb, in_=bv_d.to_broadcast([128, D2]))
        for c in range(CT):
            nc.sync.dma_start(out=xh8_sb[:, c, :],
                              in_=xh8_d[c * 128:(c + 1) * 128, :])
        wos = wstage.tile([128, PAIRS, C_], F32, tag="ws", name="wos")
        nc.sync.dma_start(out=wos, in_=wo_d.rearrange("(j p) o -> p j o", p=128))
        nc.vector.tensor_copy(wo_sb, wos)

        for w_sb, b_sb, dest in (
            (wq_sb, bq_sb, qT_sb),
            (wk_sb, bk_sb, kT_sb),
        ):
            for nb in range(NPB):
                pss = [prj_ps.tile([128, PB], F32, tag=f"prj{j}",
                                   name=f"prj_{dest.tensor.name}_{nb}_{j}")
                       for j in range(PAIRS)]
                for c in range(CT):
                    for j in range(PAIRS):
                        nc.tensor.matmul(
                            pss[j],
                            w_sb[:, c, j * 128:(j + 1) * 128],
                            xTnb[nb][:, c, :],
                            start=(c == 0),
                            stop=(c == CT - 1),
                        )
                for j in range(PAIRS):
                    nc.vector.tensor_scalar_add(
                        dest[:, j, nb * PB:(nb + 1) * PB], pss[j], b_sb[:, j, :]
                    )

        # V computed directly with keys on partitions (lhsT = x^T chunk)
        for kt in range(NKT):
            nb, off = divmod(kt, PB // 128)
            off *= 128
            vp = v_ps.tile([128, D2], F32, tag="vp")
            for c in range(CT):
                nc.tensor.matmul(
                    vp,
                    xTnb[nb][:, c, off:off + 128],
                    wv_sb[:, c, :],
                    start=(c == 0),
                    stop=(c == CT - 1),
                )
            nc.vector.tensor_add(
                v2_sb[:, :, kt, 0:HEAD_DIM],
                vp.rearrange("p (h d) -> p h d", h=HPC_),
                vbias_sb.rearrange("p (h d) -> p h d", h=HPC_),
            )
        onescol = wstage.tile([128, HPC_, NKT, 1], F32)
        nc.vector.memset(onescol, WSCALE)
        nc.vector.tensor_copy(v2_sb[:, :, :, HEAD_DIM:HEAD_DIM + 1], onescol)

    # per-qb ctx tiles so the out-projection of early query blocks does not
    # serialize on the last block's divisions (whole-tile dependency)
    ctxT2_nq = [projpool.tile([128, PAIRS, QB], F32R, name=f"ctxT2_{nq}")
                for nq in range(NQB)]

    # --- main loop (phase 2) ---
    ptpool = stack.enter_context(tc.tile_pool(name="ptpool", bufs=9))
    spool = stack.enter_context(tc.tile_pool(name="spool", bufs=2))
    simsb = stack.enter_context(tc.tile_pool(name="simsb", bufs=2))
    smallpool = stack.enter_context(tc.tile_pool(name="smallpool", bufs=2))
    mpool = (stack.enter_context(tc.tile_pool(name="mpool", bufs=2))
             if mask_d is not None else None)

    with tc.tile_pool(name="simpsum", bufs=2, space="PSUM") as simp, \
         tc.tile_pool(name="scpsum", bufs=1, space="PSUM") as scp, \
         tc.tile_pool(name="ctxpsum", bufs=1, space="PSUM") as ctxp:

        def emit_ctx(ctx_ps, kt, pt_pairs):
            for j in range(PAIRS):
                for hi in range(2):
                    nc.tensor.matmul(
                        ctx_ps[2 * j + hi],
                        v2_sb[:, 2 * j + hi, kt, :],
                        pt_pairs[j][:, hi, :],
                        start=(kt == 0),
                        stop=(kt == NKT - 1),
                        skip_group_check=True,
                    )

        def emit_division_head(qb, ctx_ps, h):
            # short chain: single-op approx reciprocal of the denominator
            # row (DVE, PSUM read), GpSimd partition-0 broadcast, DVE mul.
            j, hi = divmod(h, 2)
            r0c = smallpool.tile([1, QB], F32, tag=f"r0c{h % 2}",
                                 name=f"r0c_{qb}_{h}")
            nc.vector.tensor_copy(r0c, ctx_ps[h][HEAD_DIM:HEAD_DIM + 1, :])
            r0 = smallpool.tile([1, QB], F32, tag=f"r0{h % 2}",
                                name=f"r0_{qb}_{h}")
            nc.vector.reciprocal_approx_fast(out=r0, in_=r0c)
            rb = smallpool.tile([HEAD_DIM, QB], F32, tag="rb",
                                name=f"rb_{qb}_{h}")
            nc.gpsimd.partition_broadcast(rb, r0, channels=HEAD_DIM)
            nc.vector.tensor_mul(
                ctxT2_nq[qb][hi * 64:hi * 64 + 64, j, :],
                ctx_ps[h][0:HEAD_DIM, :],
                rb,
            )

        def emit_division(qb, ctx_ps):
            for h in range(HPC_):
                emit_division_head(qb, ctx_ps, h)

        prev_div = None
        for qb in range(NQB):
            ctx_ps = [ctxp.tile([HEAD_DIM + 1, QB], F32, tag=f"ctx{h}",
                                name=f"ctx_{qb}_{h}")
                      for h in range(HPC_)]
            pending = []
            for kt in range(NKT):
                sp = simp.tile([128, QB], F32, tag="sim")
                for cp in range(CP):
                    nc.tensor.matmul(
                        sp,
                        xh8_sb[:, 2 * cp:2 * cp + 2, kt * 128:(kt + 1) * 128],
                        xh8_sb[:, 2 * cp:2 * cp + 2, qb * QB:(qb + 1) * QB],
                        start=(cp == 0),
                        stop=(cp == CP - 1),
                        perf_mode=DOUBLE_ROW,
                    )
                # HW allows only one PSUM operand per DVE op: stage sim in
                # SBUF (alternate ACT/DVE to balance engine load)
                sim_t = simsb.tile([128, QB], F32, tag="simsb")
                if kt % 2 == 0:
                    nc.scalar.activation(out=sim_t, in_=sp, func=ACT_COPY)
                else:
                    nc.vector.tensor_copy(sim_t, sp)
                sim_in = sim_t.unsqueeze(1).to_broadcast([128, 2, QB])
                if mask_d is not None:
                    m_sb = mpool.tile([128, QB], F32, tag="msk")
                    nc.sync.dma_start(
                        out=m_sb,
                        in_=mask_d[kt * 128:(kt + 1) * 128, qb * QB:(qb + 1) * QB],
                    )
                pt_pairs = []
                for j in range(PAIRS):
                    sc_t = scp.tile([128, 2, QB], F32, tag="scp")
                    for hi in range(2):
                        pr = slice(hi * 64, hi * 64 + 64)
                        nc.tensor.matmul(
                            sc_t[:, hi, :],
                            kT_sb[pr, j, kt * 128:(kt + 1) * 128],
                            qT_sb[pr, j, qb * QB:(qb + 1) * QB],
                            start=True,
                            stop=True,
                        )
                    # interleave ctx matmuls (lagged) between the two score
                    # pairs so the PE has work while the DVE runs the sub
                    if j == 0 and pending and len(pending) > LAG:
                        k0, p0 = pending.pop(0)
                        emit_ctx(ctx_ps, k0, p0)
                    s_t = spool.tile([128, 2, QB], F32, tag="s")
                    nc.vector.tensor_sub(s_t, sc_t, sim_in)
                    if mask_d is not None:
                        nc.vector.tensor_sub(
                            s_t, s_t, m_sb.unsqueeze(1).to_broadcast([128, 2, QB]))
                    pt = ptpool.tile([128, 2, QB], F32R, tag="pt")
                    nc.scalar.activation(out=pt, in_=s_t, func=ACT_EXP,
                                         scale=EXP_DESCALE)
                    pt_pairs.append(pt)
                pending.append((kt, pt_pairs))
                # previous block's divisions, spread over the kt loop on GpSimd
                if prev_div is not None and kt >= 2 and (kt - 2) % 3 == 0:
                    h = (kt - 2) // 3
                    if h < HPC_:
                        emit_division_head(prev_div[0], prev_div[1], h)
                        if h == HPC_ - 1:
                            prev_div = None
            for k0, p0 in pending:
                emit_ctx(ctx_ps, k0, p0)
            if prev_div is not None:
                done = max(0, (NKT - 1 - 2) // 3 + 1) if NKT > 2 else 0
                for h in range(min(done, HPC_), HPC_):
                    emit_division_head(prev_div[0], prev_div[1], h)
                prev_div = None
            prev_div = (qb, ctx_ps)
        emit_division(*prev_div)

    # --- out-projection (phase 3) ---
    with tc.tile_pool(name="outpsum", bufs=4, space="PSUM") as outp, \
         tc.tile_pool(name="outstg", bufs=4) as outstg:
        QT_PER = QB // 128
        for qt in range(S_ // 128):
            for ob in range(NOB):
                op = outp.tile([128, OB_W], F32, tag="op")
                for j in range(PAIRS):
                    nc.tensor.matmul(
                        op,
                        ctxT2_nq[qt // QT_PER][
                            :, j, (qt % QT_PER) * 128:(qt % QT_PER + 1) * 128],
                        wo_sb[:, j, ob * OB_W:(ob + 1) * OB_W],
                        start=(j == 0),
                        stop=(j == PAIRS - 1),
                    )
                ostg = outstg.tile([128, OB_W], F32, tag="ostg")
                if (qt + ob) % 2 == 0:
                    nc.scalar.activation(out=ostg, in_=op, func=ACT_COPY)
                else:
                    nc.vector.tensor_copy(ostg, op)
                nc.sync.dma_start(
                    out=out_d[qt * 128:(qt + 1) * 128, ob * OB_W:(ob + 1) * OB_W],
                    in_=ostg,
                )

    stack.close()


def build_nc(*, S_=S, C_=HIDDEN, HPC_=HPC, QB=512, with_mask=False,
             enable_asserts=False):
    nc = bacc.Bacc(
        "TRN2", target_bir_lowering=False, debug=False,
        enable_asserts=enable_asserts,
    )
    D2 = HPC_ * HEAD_DIM
    aps = {}
    aps["xT"] = nc.dram_tensor("xT", [C_, S_], BF16, kind="ExternalInput").ap()
    aps["xh8"] = nc.dram_tensor("xh8", [C_, S_], FP8, kind="ExternalInput").ap()
    for n in ("wq", "wk", "wv"):
        aps[n] = nc.dram_tensor(n, [C_, D2], F32, kind="ExternalInput").ap()
    aps["wo"] = nc.dram_tensor("wo", [D2, C_], F32, kind="ExternalInput").ap()
    for n in ("bq", "bk"):
        aps[n] = nc.dram_tensor(n, [D2, 1], F32, kind="ExternalInput").ap()
    aps["bv"] = nc.dram_tensor("bv", [1, D2], F32, kind="ExternalInput").ap()
    if with_mask:
        aps["maskadd"] = nc.dram_tensor(
            "maskadd", [S_, S_], F32, kind="ExternalInput").ap()
    aps["out"] = nc.dram_tensor("out", [S_, C_], F32, kind="ExternalOutput").ap()

    with tile.TileContext(nc) as tc:
        emit_kernel(tc, aps, S_=S_, C_=C_, HPC_=HPC_, QB=QB)
    nc.compile()
    return nc


def _q8(a):
    return a.astype(NP_FP8)


def host_prepare(x, attn_mask, Wq, bq, Wk, bk, Wv, bv, Wo, bo, *,
                 S_=S, C_=HIDDEN, HPC_=HPC, n_cores=N_CORES):
    """Build the per-core input maps. Returns (in_maps, with_mask)."""
    x = np.asarray(x, np.float32)
    B_ = x.shape[0]
    groups = n_cores // B_
    Wq = np.asarray(Wq, np.float32); Wk = np.asarray(Wk, np.float32)
    Wv = np.asarray(Wv, np.float32); Wo = np.asarray(Wo, np.float32)
    bq = np.asarray(bq, np.float32); bk = np.asarray(bk, np.float32)
    bv = np.asarray(bv, np.float32)

    inv_sqrt_d = 1.0 / math.sqrt(HEAD_DIM)
    # weights carry the 256x matched scale so scores_psum = 65536 * s
    WqT = np.ascontiguousarray(Wq.T * (inv_sqrt_d * WSCALE)).astype(np.float32)
    WkT = np.ascontiguousarray(Wk.T * WSCALE).astype(np.float32)
    WvT = np.ascontiguousarray(Wv.T * WSCALE).astype(np.float32)
    WoT = np.ascontiguousarray(Wo.T)                 # [C(c), C(o)]
    bq_s = bq * (inv_sqrt_d * WSCALE)
    bk_s = bk * WSCALE
    bv_s = bv * WSCALE

    mask = np.asarray(attn_mask)
    with_mask = bool(mask.any())
    maskadd = None
    if with_mask:
        # s_t -= maskadd; masked positions get -1e20/65536 pre-exp -> 0
        maskadd = np.where(mask, np.float32(1e20), np.float32(0.0)).astype(np.float32)
        maskadd = np.ascontiguousarray(maskadd.T)  # [k, q]

    in_maps = []
    per_b = {}
    for core in range(n_cores):
        b, g = divmod(core, groups)
        if b not in per_b:
            xb = x[b]                               # [S, C]
            xT = np.ascontiguousarray(xb.T)         # [C, S]
            norms = np.linalg.norm(xb, axis=1)      # [S]
            scale = (math.sqrt(GAMMA) * WSCALE /
                     np.maximum(norms, 1e-12)).astype(np.float32)
            xh8 = _q8(xT * scale[None, :])
            xT16 = np.ascontiguousarray(xT.astype(ml_dtypes.bfloat16))
            per_b[b] = (xT16, np.ascontiguousarray(xh8))
        xT, xh8 = per_b[b]
        ch = slice(g * HPC_ * HEAD_DIM, (g + 1) * HPC_ * HEAD_DIM)
        m = {
            "xT": xT, "xh8": xh8,
            "wq": np.ascontiguousarray(WqT[:, ch]),
            "wk": np.ascontiguousarray(WkT[:, ch]),
            "wv": np.ascontiguousarray(WvT[:, ch]),
            "wo": np.ascontiguousarray(WoT[ch, :]),
            "bq": np.ascontiguousarray(bq_s[ch]).reshape(-1, 1),
            "bk": np.ascontiguousarray(bk_s[ch]).reshape(-1, 1),
            "bv": np.ascontiguousarray(bv_s[ch]).reshape(1, -1),
        }
        if with_mask:
            m["maskadd"] = maskadd
        in_maps.append(m)
    return in_maps, with_mask


_NC_CACHE = {}


def _get_nc(with_mask):
    key = with_mask
    if key not in _NC_CACHE:
        _NC_CACHE[key] = build_nc(with_mask=with_mask)
    return _NC_CACHE[key]


LAST_RESULTS = None


def kernel(**inputs):
    global LAST_RESULTS
    in_maps, with_mask = host_prepare(
        inputs["x"], inputs["attn_mask"],
        inputs["Wq"], inputs["bq"], inputs["Wk"], inputs["bk"],
        inputs["Wv"], inputs["bv"], inputs["Wo"], inputs["bo"],
    )
    nc = _get_nc(with_mask)
    res = run_bass_kernel_spmd(nc, in_maps, core_ids=list(range(N_CORES)))
    LAST_RESULTS = res
    bo = np.asarray(inputs["bo"], np.float32)
    out = np.zeros((B, S, HIDDEN), np.float32)
    groups = N_CORES // B
    for core in range(N_CORES):
        b = core // groups
        out[b] += res.results[core]["out"]
    out += bo[None, None, :]
    return out


# revision 53
# speedup vs baseline: 1.0147x; 1.0138x over previous
"""DiversityAttention on 8 TRN2 NeuronCores (Bass/Tile).

Sharding: data-parallel over batch (B=2) x tensor-parallel over heads
(16 heads -> 4 groups of 4). core = (b, g), b = core // 4, g = core % 4.
Each core computes full attention for its 4 heads over its batch and a
partial out-projection [S, HIDDEN]; the host sums the 4 partials per
batch and adds bo.

All big matmuls run as fp8e4 DoubleRow (0.25x the fp32r cost) where
precision allows; q/k/v projections use a 3-term fp8 residual
decomposition (W8@x8 + W8@xr8 + Wr8@x8 ~ W@x to ~0.1%).

Scale conventions (host-side):
  wq8/wqr8 = fp8(256 * Wq^T / sqrt(dh)), bq' = 256*bq/sqrt(dh)
  wk8/wkr8 = fp8(256 * Wk^T),            bk' = 256*bk
  wv8/wvr8 = fp8(256 * Wv^T),            bv' = 256*bv
  x8/xr8   = fp8(x^T) + fp8 residual
  xh8      = fp8(sqrt(gamma) * 256 * x^T / max(||x||, eps))
so on device:
  qT_sb = 256*q, kT_sb = 256*k   -> scores_psum = 65536 * s
  sim_psum = 65536 * gamma * sim -> s_t = 65536*(s - gamma*sim)
  P = exp(s_t / 65536)  (activation scale)
  v2 = [256 | 256*v] per head (ones col at 0 for the denominator row);
  ctx_psum row 0 = 256*sum(P), rows 1..64 = 256*sum(v P): ratio exact.
Division runs fully on GpSimd: partition_broadcast(denom row 0) ->
reciprocal -> multiply, keeping the DVE queue free for the score-sim
subtractions.
"""

import math
import os
import sys

import numpy as np

for _p in ("/opt/trn_rl_repo",):
    if _p not in sys.path and os.path.isdir(_p):
        sys.path.insert(0, _p)

os.environ.setdefault("MYCRO_LOCAL_CACHE", "1")

import ml_dtypes

import concourse.bass as bass
import concourse.tile as tile
from concourse import bacc, mybir
from concourse.bass_utils import run_bass_kernel_spmd


def _install_ntff_hook():
    """Provide antenv.axon_hooks (NTFF profiling registry) if the image
    lacks it, mirroring trn_agent_boot's ctypes hook. No-op on failure."""
    try:
        import antenv.axon_hooks  # noqa: F401
        return
    except ImportError:
        pass
    try:
        import contextlib
        import ctypes
        import types

        so_path = "/opt/axon/libaxon_pjrt.so"
        if not os.path.exists(so_path):
            return
        lib = ctypes.CDLL(so_path)
        if not hasattr(lib, "axon_start_nrt_profile"):
            return
        lib.axon_start_nrt_profile.argtypes = [
            ctypes.POINTER(ctypes.c_int64), ctypes.c_size_t]
        lib.axon_start_nrt_profile.restype = ctypes.c_int64
        lib.axon_stop_nrt_profile.argtypes = [ctypes.c_char_p]
        lib.axon_stop_nrt_profile.restype = ctypes.c_int64

        @contextlib.contextmanager
        def _hook(output_dir, device_ids):
            import jax
            jax.devices()
            if device_ids:
                ids = (ctypes.c_int64 * len(device_ids))(*device_ids)
                rc = lib.axon_start_nrt_profile(ids, len(device_ids))
            else:
                rc = lib.axon_start_nrt_profile(None, 0)
            if rc != 0:
                raise RuntimeError(f"axon_start_nrt_profile rc={rc}")
            try:
                yield
            finally:
                n = lib.axon_stop_nrt_profile(str(output_dir).encode())
                print(f"ntff profile: {n} file(s) -> {output_dir}",
                      file=sys.stderr)

        mod = types.ModuleType("antenv.axon_hooks")
        _state = {"hook": _hook}
        mod.set_axon_ntff_profile_hook = lambda h: _state.__setitem__("hook", h)
        mod.get_axon_ntff_profile_hook = lambda: _state["hook"]
        sys.modules["antenv.axon_hooks"] = mod
        import antenv
        antenv.axon_hooks = mod
    except Exception:
        pass


_install_ntff_hook()

F32 = mybir.dt.float32
F32R = mybir.dt.float32r
BF16 = mybir.dt.bfloat16
FP8 = mybir.dt.float8e4
NP_FP8 = ml_dtypes.float8_e4m3
ACT_EXP = mybir.ActivationFunctionType.Exp
ACT_COPY = mybir.ActivationFunctionType.Copy
DOUBLE_ROW = mybir.MatmulPerfMode.DoubleRow

# Problem constants (hardcoded per contract).
HIDDEN = 1024
HEADS = 16
HEAD_DIM = 64
GAMMA = 0.5
B, S = 2, 2048
N_CORES = 8
GROUPS = N_CORES // B  # head groups per batch
HPC = HEADS // GROUPS  # heads per core
LAG = 3  # kt software-pipeline lag between exp and ctx matmul
WSCALE = 256.0
EXP_DESCALE = 1.0 / (WSCALE * WSCALE)


def emit_kernel(tc, aps, *, S_, C_, HPC_, QB):
    """Emit the per-core kernel. aps: dict of dram APs."""
    nc = tc.nc
    CT = C_ // 128          # contraction tiles over hidden
    CP = CT // 2            # contraction pairs for fp8 DoubleRow
    PAIRS = HPC_ // 2       # head pairs (128-channel chunks)
    D2 = HPC_ * HEAD_DIM
    NKT = S_ // 128         # key tiles
    NQB = S_ // QB          # query blocks
    PB = min(512, S_)       # projection free-block width
    NPB = S_ // PB
    KPB = PB // 128         # key tiles per nb block
    OB_W = min(512, C_)     # out-projection free-block width
    NOB = C_ // OB_W

    xh8_d = aps["xh8"]; xT_d = aps["xT"]
    wq_d = aps["wq"]; wk_d = aps["wk"]; wv_d = aps["wv"]; wo_d = aps["wo"]
    bq_d = aps["bq"]; bk_d = aps["bk"]; bv_d = aps["bv"]
    out_d = aps["out"]
    mask_d = aps.get("maskadd")

    from contextlib import ExitStack
    stack = ExitStack()
    consts = stack.enter_context(tc.tile_pool(name="consts", bufs=1))
    xpool = stack.enter_context(tc.tile_pool(name="xpool", bufs=1))
    projpool = stack.enter_context(tc.tile_pool(name="projpool", bufs=1))

    wo_sb = consts.tile([128, PAIRS, C_], F32R)
    xh8_sb = xpool.tile([128, CT, S_], FP8)

    # projections (fp32r, pre-scaled by 256; V in [keys, 1+dims] layout
    # with the 256-valued denominator column at position 0)
    qT_sb = projpool.tile([128, PAIRS, S_], F32R)
    kT_sb = projpool.tile([128, PAIRS, S_], F32R)
    v2_sb = projpool.tile([128, HPC_, NKT, HEAD_DIM + 1], F32R)

    with tc.tile_pool(name="xtpool", bufs=1) as xtpool, \
         tc.tile_pool(name="wstage", bufs=1) as wstage, \
         tc.tile_pool(name="wpool", bufs=1) as wpool, \
         tc.tile_pool(name="ph1psum", bufs=2, space="PSUM") as prj_ps, \
         tc.tile_pool(name="vpsum", bufs=2, space="PSUM") as v_ps:
        # x^T arrives as bf16 (host cast); per-nb tiles so the first
        # projection block starts after ~1MB of DMA instead of 4MB
        xTnb = [xtpool.tile([128, CT, PB], BF16, name=f"xT_{nb}")
                for nb in range(NPB)]
        # weights arrive as bf16 from the host: DMA straight in, no rounding
        wq_sb = wpool.tile([128, CT, D2], BF16)
        wk_sb = wpool.tile([128, CT, D2], BF16)
        wv_sb = wpool.tile([128, CT, D2], BF16)
        nc.sync.dma_start(out=wq_sb,
                          in_=wq_d.rearrange("(t p) m -> p t m", p=128))
        xT_r = xT_d.rearrange("(t p) m -> p t m", p=128)
        for nb in range(NPB):
            nc.sync.dma_start(out=xTnb[nb],
                              in_=xT_r[:, :, nb * PB:(nb + 1) * PB])
        for w_sb, w_d in ((wk_sb, wk_d), (wv_sb, wv_d)):
            nc.sync.dma_start(out=w_sb,
                              in_=w_d.rearrange("(t p) m -> p t m", p=128))
        bq_sb = wpool.tile([128, PAIRS, 1], F32)
        bk_sb = wpool.tile([128, PAIRS, 1], F32)
        for b_sb, b_d in ((bq_sb, bq_d), (bk_sb, bk_d)):
            nc.sync.dma_start(
                out=b_sb, in_=b_d.rearrange("(j p) one -> p j one", p=128))
        vbias_sb = wpool.tile([128, D2], F32)
        nc.sync.dma_start(out=vbias_sb, in_=bv_d.to_broadcast([128, D2]))
        for c in range(CT):
            nc.sync.dma_start(out=xh8_sb[:, c, :],
                              in_=xh8_d[c * 128:(c + 1) * 128, :])
        wos = wstage.tile([128, PAIRS, C_], F32, tag="ws", name="wos")
        nc.sync.dma_start(out=wos, in_=wo_d.rearrange("(j p) o -> p j o", p=128))
        nc.vector.tensor_copy(wo_sb, wos)

        for w_sb, b_sb, dest in (
            (wq_sb, bq_sb, qT_sb),
            (wk_sb, bk_sb, kT_sb),
        ):
            for nb in range(NPB):
                pss = [prj_ps.tile([128, PB], F32, tag=f"prj{j}",
                                   name=f"prj_{dest.tensor.name}_{nb}_{j}")
                       for j in range(PAIRS)]
                for c in range(CT):
                    for j in range(PAIRS):
                        nc.tensor.matmul(
                            pss[j],
                            w_sb[:, c, j * 128:(j + 1) * 128],
                            xTnb[nb][:, c, :],
                            start=(c == 0),
                            stop=(c == CT - 1),
                        )
                for j in range(PAIRS):
                    nc.vector.tensor_scalar_add(
                        dest[:, j, nb * PB:(nb + 1) * PB], pss[j], b_sb[:, j, :]
                    )

        # V computed directly with keys on partitions (lhsT = x^T chunk)
        for kt in range(NKT):
            nb, off = divmod(kt, PB // 128)
            off *= 128
            vp = v_ps.tile([128, D2], F32, tag="vp")
            for c in range(CT):
                nc.tensor.matmul(
                    vp,
                    xTnb[nb][:, c, off:off + 128],
                    wv_sb[:, c, :],
                    start=(c == 0),
                    stop=(c == CT - 1),
                )
            nc.vector.tensor_add(
                v2_sb[:, :, kt, 0:HEAD_DIM],
                vp.rearrange("p (h d) -> p h d", h=HPC_),
                vbias_sb.rearrange("p (h d) -> p h d", h=HPC_),
            )
        onescol = wstage.tile([128, HPC_, NKT, 1], F32)
        nc.vector.memset(onescol, WSCALE)
        nc.vector.tensor_copy(v2_sb[:, :, :, HEAD_DIM:HEAD_DIM + 1], onescol)

    # per-qb ctx tiles so the out-projection of early query blocks does not
    # serialize on the last block's divisions (whole-tile dependency)
    ctxT2_nq = [projpool.tile([128, PAIRS, QB], F32R, name=f"ctxT2_{nq}")
                for nq in range(NQB)]

    # --- main loop (phase 2) ---
    ptpool = stack.enter_context(tc.tile_pool(name="ptpool", bufs=9))
    spool = stack.enter_context(tc.tile_pool(name="spool", bufs=2))
    simsb = stack.enter_context(tc.tile_pool(name="simsb", bufs=2))
    smallpool = stack.enter_context(tc.tile_pool(name="smallpool", bufs=2))
    mpool = (stack.enter_context(tc.tile_pool(name="mpool", bufs=2))
             if mask_d is not None else None)

    with tc.tile_pool(name="simpsum", bufs=2, space="PSUM") as simp, \
         tc.tile_pool(name="scpsum", bufs=1, space="PSUM") as scp, \
         tc.tile_pool(name="ctxpsum", bufs=1, space="PSUM") as ctxp:

        def emit_ctx(ctx_ps, kt, pt_pairs):
            for j in range(PAIRS):
                for hi in range(2):
                    nc.tensor.matmul(
                        ctx_ps[2 * j + hi],
                        v2_sb[:, 2 * j + hi, kt, :],
                        pt_pairs[j][:, hi, :],
                        start=(kt == 0),
                        stop=(kt == NKT - 1),
                        skip_group_check=True,
                    )

        def emit_division_head(qb, ctx_ps, h):
            # short chain: single-op approx reciprocal of the denominator
            # row (DVE, PSUM read), GpSimd partition-0 broadcast, DVE mul.
            j, hi = divmod(h, 2)
            r0c = smallpool.tile([1, QB], F32, tag=f"r0c{h % 2}",
                                 name=f"r0c_{qb}_{h}")
            nc.vector.tensor_copy(r0c, ctx_ps[h][HEAD_DIM:HEAD_DIM + 1, :])
            r0 = smallpool.tile([1, QB], F32, tag=f"r0{h % 2}",
                                name=f"r0_{qb}_{h}")
            nc.vector.reciprocal_approx_fast(out=r0, in_=r0c)
            rb = smallpool.tile([HEAD_DIM, QB], F32, tag="rb",
                                name=f"rb_{qb}_{h}")
            nc.gpsimd.partition_broadcast(rb, r0, channels=HEAD_DIM)
            nc.vector.tensor_mul(
                ctxT2_nq[qb][hi * 64:hi * 64 + 64, j, :],
                ctx_ps[h][0:HEAD_DIM, :],
                rb,
            )

        def emit_division(qb, ctx_ps):
            for h in range(HPC_):
                emit_division_head(qb, ctx_ps, h)

        prev_div = None
        for qb in range(NQB):
            ctx_ps = [ctxp.tile([HEAD_DIM + 1, QB], F32, tag=f"ctx{h}",
                                name=f"ctx_{qb}_{h}")
                      for h in range(HPC_)]
            pending = []
            for kt in range(NKT):
                sp = simp.tile([128, QB], F32, tag="sim")
                for cp in range(CP):
                    nc.tensor.matmul(
                        sp,
                        xh8_sb[:, 2 * cp:2 * cp + 2, kt * 128:(kt + 1) * 128],
                        xh8_sb[:, 2 * cp:2 * cp + 2, qb * QB:(qb + 1) * QB],
                        start=(cp == 0),
                        stop=(cp == CP - 1),
                        perf_mode=DOUBLE_ROW,
                    )
                # HW allows only one PSUM operand per DVE op: stage sim in
                # SBUF (alternate ACT/DVE to balance engine load)
                sim_t = simsb.tile([128, QB], F32, tag="simsb")
                if kt % 2 == 0:
                    nc.scalar.activation(out=sim_t, in_=sp, func=ACT_COPY)
                else:
                    nc.vector.tensor_copy(sim_t, sp)
                sim_in = sim_t.unsqueeze(1).to_broadcast([128, 2, QB])
                if mask_d is not None:
                    m_sb = mpool.tile([128, QB], F32, tag="msk")
                    nc.sync.dma_start(
                        out=m_sb,
                        in_=mask_d[kt * 128:(kt + 1) * 128, qb * QB:(qb + 1) * QB],
                    )
                pt_pairs = []
                for j in range(PAIRS):
                    sc_t = scp.tile([128, 2, QB], F32, tag="scp")
                    for hi in range(2):
                        pr = slice(hi * 64, hi * 64 + 64)
                        nc.tensor.matmul(
                            sc_t[:, hi, :],
                            kT_sb[pr, j, kt * 128:(kt + 1) * 128],
                            qT_sb[pr, j, qb * QB:(qb + 1) * QB],
                            start=True,
                            stop=True,
                        )
                    # interleave ctx matmuls (lagged) between the two score
                    # pairs so the PE has work while the DVE runs the sub
                    if j == 0 and pending and len(pending) > LAG:
                        k0, p0 = pending.pop(0)
                        emit_ctx(ctx_ps, k0, p0)
                    s_t = spool.tile([128, 2, QB], F32, tag="s")
                    nc.vector.tensor_sub(s_t, sc_t, sim_in)
                    if mask_d is not None:
                        nc.vector.tensor_sub(
                            s_t, s_t, m_sb.unsqueeze(1).to_broadcast([128, 2, QB]))
                    pt = ptpool.tile([128, 2, QB], F32R, tag="pt")
                    nc.scalar.activation(out=pt, in_=s_t, func=ACT_EXP,
                                         scale=EXP_DESCALE)
                    pt_pairs.append(pt)
                pending.append((kt, pt_pairs))
                # previous block's divisions, spread over the kt loop on GpSimd
                if prev_div is not None and kt >= 2 and (kt - 2) % 3 == 0:
                    h = (kt - 2) // 3
                    if h < HPC_:
                        emit_division_head(prev_div[0], prev_div[1], h)
                        if h == HPC_ - 1:
                            prev_div = None
            for k0, p0 in pending:
                emit_ctx(ctx_ps, k0, p0)
            if prev_div is not None:
                done = max(0, (NKT - 1 - 2) // 3 + 1) if NKT > 2 else 0
                for h in range(min(done, HPC_), HPC_):
                    emit_division_head(prev_div[0], prev_div[1], h)
                prev_div = None
            prev_div = (qb, ctx_ps)
        emit_division(*prev_div)

    # --- out-projection (phase 3) ---
    with tc.tile_pool(name="outpsum", bufs=4, space="PSUM") as outp, \
         tc.tile_pool(name="outstg", bufs=4) as outstg:
        QT_PER = QB // 128
        for qt in range(S_ // 128):
            for ob in range(NOB):
                op = outp.tile([128, OB_W], F32, tag="op")
                for j in range(PAIRS):
                    nc.tensor.matmul(
                        op,
                        ctxT2_nq[qt // QT_PER][
                            :, j, (qt % QT_PER) * 128:(qt % QT_PER + 1) * 128],
                        wo_sb[:, j, ob * OB_W:(ob + 1) * OB_W],
                        start=(j == 0),
                        stop=(j == PAIRS - 1),
                    )
                ostg = outstg.tile([128, OB_W], BF16, tag="ostg")
                if (qt + ob) % 2 == 0:
                    nc.scalar.activation(out=ostg, in_=op, func=ACT_COPY)
                else:
                    nc.vector.tensor_copy(ostg, op)
                nc.sync.dma_start(
                    out=out_d[qt * 128:(qt + 1) * 128, ob * OB_W:(ob + 1) * OB_W],
                    in_=ostg,
                )

    stack.close()


def build_nc(*, S_=S, C_=HIDDEN, HPC_=HPC, QB=512, with_mask=False,
             enable_asserts=False):
    nc = bacc.Bacc(
        "TRN2", target_bir_lowering=False, debug=False,
        enable_asserts=enable_asserts,
    )
    D2 = HPC_ * HEAD_DIM
    aps = {}
    aps["xT"] = nc.dram_tensor("xT", [C_, S_], BF16, kind="ExternalInput").ap()
    aps["xh8"] = nc.dram_tensor("xh8", [C_, S_], FP8, kind="ExternalInput").ap()
    for n in ("wq", "wk", "wv"):
        aps[n] = nc.dram_tensor(n, [C_, D2], BF16, kind="ExternalInput").ap()
    aps["wo"] = nc.dram_tensor("wo", [D2, C_], F32, kind="ExternalInput").ap()
    for n in ("bq", "bk"):
        aps[n] = nc.dram_tensor(n, [D2, 1], F32, kind="ExternalInput").ap()
    aps["bv"] = nc.dram_tensor("bv", [1, D2], F32, kind="ExternalInput").ap()
    if with_mask:
        aps["maskadd"] = nc.dram_tensor(
            "maskadd", [S_, S_], F32, kind="ExternalInput").ap()
    aps["out"] = nc.dram_tensor("out", [S_, C_], BF16, kind="ExternalOutput").ap()

    with tile.TileContext(nc) as tc:
        emit_kernel(tc, aps, S_=S_, C_=C_, HPC_=HPC_, QB=QB)
    nc.compile()
    return nc


def _q8(a):
    return a.astype(NP_FP8)


def host_prepare(x, attn_mask, Wq, bq, Wk, bk, Wv, bv, Wo, bo, *,
                 S_=S, C_=HIDDEN, HPC_=HPC, n_cores=N_CORES):
    """Build the per-core input maps. Returns (in_maps, with_mask)."""
    x = np.asarray(x, np.float32)
    B_ = x.shape[0]
    groups = n_cores // B_
    Wq = np.asarray(Wq, np.float32); Wk = np.asarray(Wk, np.float32)
    Wv = np.asarray(Wv, np.float32); Wo = np.asarray(Wo, np.float32)
    bq = np.asarray(bq, np.float32); bk = np.asarray(bk, np.float32)
    bv = np.asarray(bv, np.float32)

    inv_sqrt_d = 1.0 / math.sqrt(HEAD_DIM)
    # weights carry the 256x matched scale so scores_psum = 65536 * s;
    # cast to bf16 on host (device DMAs them straight into SBUF)
    WqT = np.ascontiguousarray(
        (Wq.T * (inv_sqrt_d * WSCALE)).astype(ml_dtypes.bfloat16))
    WkT = np.ascontiguousarray((Wk.T * WSCALE).astype(ml_dtypes.bfloat16))
    WvT = np.ascontiguousarray((Wv.T * WSCALE).astype(ml_dtypes.bfloat16))
    WoT = np.ascontiguousarray(Wo.T)                 # [C(c), C(o)]
    bq_s = bq * (inv_sqrt_d * WSCALE)
    bk_s = bk * WSCALE
    bv_s = bv * WSCALE

    mask = np.asarray(attn_mask)
    with_mask = bool(mask.any())
    maskadd = None
    if with_mask:
        # s_t -= maskadd; masked positions get -1e20/65536 pre-exp -> 0
        maskadd = np.where(mask, np.float32(1e20), np.float32(0.0)).astype(np.float32)
        maskadd = np.ascontiguousarray(maskadd.T)  # [k, q]

    in_maps = []
    per_b = {}
    for core in range(n_cores):
        b, g = divmod(core, groups)
        if b not in per_b:
            xb = x[b]                               # [S, C]
            xT = np.ascontiguousarray(xb.T)         # [C, S]
            norms = np.linalg.norm(xb, axis=1)      # [S]
            scale = (math.sqrt(GAMMA) * WSCALE /
                     np.maximum(norms, 1e-12)).astype(np.float32)
            xh8 = _q8(xT * scale[None, :])
            xT16 = np.ascontiguousarray(xT.astype(ml_dtypes.bfloat16))
            per_b[b] = (xT16, np.ascontiguousarray(xh8))
        xT, xh8 = per_b[b]
        ch = slice(g * HPC_ * HEAD_DIM, (g + 1) * HPC_ * HEAD_DIM)
        m = {
            "xT": xT, "xh8": xh8,
            "wq": np.ascontiguousarray(WqT[:, ch]),
            "wk": np.ascontiguousarray(WkT[:, ch]),
            "wv": np.ascontiguousarray(WvT[:, ch]),
            "wo": np.ascontiguousarray(WoT[ch, :]),
            "bq": np.ascontiguousarray(bq_s[ch]).reshape(-1, 1),
            "bk": np.ascontiguousarray(bk_s[ch]).reshape(-1, 1),
            "bv": np.ascontiguousarray(bv_s[ch]).reshape(1, -1),
        }
        if with_mask:
            m["maskadd"] = maskadd
        in_maps.append(m)
    return in_maps, with_mask


_NC_CACHE = {}


def _get_nc(with_mask):
    key = with_mask
    if key not in _NC_CACHE:
        _NC_CACHE[key] = build_nc(with_mask=with_mask)
    return _NC_CACHE[key]


LAST_RESULTS = None


def kernel(**inputs):
    global LAST_RESULTS
    in_maps, with_mask = host_prepare(
        inputs["x"], inputs["attn_mask"],
        inputs["Wq"], inputs["bq"], inputs["Wk"], inputs["bk"],
        inputs["Wv"], inputs["bv"], inputs["Wo"], inputs["bo"],
    )
    nc = _get_nc(with_mask)
    res = run_bass_kernel_spmd(nc, in_maps, core_ids=list(range(N_CORES)))
    LAST_RESULTS = res
    bo = np.asarray(inputs["bo"], np.float32)
    out = np.zeros((B, S, HIDDEN), np.float32)
    groups = N_CORES // B
    for core in range(N_CORES):
        b = core // groups
        out[b] += res.results[core]["out"].astype(np.float32)
    out += bo[None, None, :]
    return out


# revision 58
# speedup vs baseline: 1.0679x; 1.0524x over previous
"""DiversityAttention on 8 TRN2 NeuronCores (Bass/Tile).

Sharding: data-parallel over batch (B=2) x tensor-parallel over heads
(16 heads -> 4 groups of 4). core = (b, g), b = core // 4, g = core % 4.
Each core computes full attention for its 4 heads over its batch and a
partial out-projection [S, HIDDEN]; the host sums the 4 partials per
batch and adds bo.

All big matmuls run as fp8e4 DoubleRow (0.25x the fp32r cost) where
precision allows; q/k/v projections use a 3-term fp8 residual
decomposition (W8@x8 + W8@xr8 + Wr8@x8 ~ W@x to ~0.1%).

Scale conventions (host-side):
  wq8/wqr8 = fp8(256 * Wq^T / sqrt(dh)), bq' = 256*bq/sqrt(dh)
  wk8/wkr8 = fp8(256 * Wk^T),            bk' = 256*bk
  wv8/wvr8 = fp8(256 * Wv^T),            bv' = 256*bv
  x8/xr8   = fp8(x^T) + fp8 residual
  xh8      = fp8(sqrt(gamma) * 256 * x^T / max(||x||, eps))
so on device:
  qT_sb = 256*q, kT_sb = 256*k   -> scores_psum = 65536 * s
  sim_psum = 65536 * gamma * sim -> s_t = 65536*(s - gamma*sim)
  P = exp(s_t / 65536)  (activation scale)
  v2 = [256 | 256*v] per head (ones col at 0 for the denominator row);
  ctx_psum row 0 = 256*sum(P), rows 1..64 = 256*sum(v P): ratio exact.
Division runs fully on GpSimd: partition_broadcast(denom row 0) ->
reciprocal -> multiply, keeping the DVE queue free for the score-sim
subtractions.
"""

import math
import os
import sys

import numpy as np

for _p in ("/opt/trn_rl_repo",):
    if _p not in sys.path and os.path.isdir(_p):
        sys.path.insert(0, _p)

os.environ.setdefault("MYCRO_LOCAL_CACHE", "1")

import ml_dtypes

import concourse.bass as bass
import concourse.tile as tile
from concourse import bacc, mybir
from concourse.bass_utils import run_bass_kernel_spmd


def _install_ntff_hook():
    """Provide antenv.axon_hooks (NTFF profiling registry) if the image
    lacks it, mirroring trn_agent_boot's ctypes hook. No-op on failure."""
    try:
        import antenv.axon_hooks  # noqa: F401
        return
    except ImportError:
        pass
    try:
        import contextlib
        import ctypes
        import types

        so_path = "/opt/axon/libaxon_pjrt.so"
        if not os.path.exists(so_path):
            return
        lib = ctypes.CDLL(so_path)
        if not hasattr(lib, "axon_start_nrt_profile"):
            return
        lib.axon_start_nrt_profile.argtypes = [
            ctypes.POINTER(ctypes.c_int64), ctypes.c_size_t]
        lib.axon_start_nrt_profile.restype = ctypes.c_int64
        lib.axon_stop_nrt_profile.argtypes = [ctypes.c_char_p]
        lib.axon_stop_nrt_profile.restype = ctypes.c_int64

        @contextlib.contextmanager
        def _hook(output_dir, device_ids):
            import jax
            jax.devices()
            if device_ids:
                ids = (ctypes.c_int64 * len(device_ids))(*device_ids)
                rc = lib.axon_start_nrt_profile(ids, len(device_ids))
            else:
                rc = lib.axon_start_nrt_profile(None, 0)
            if rc != 0:
                raise RuntimeError(f"axon_start_nrt_profile rc={rc}")
            try:
                yield
            finally:
                n = lib.axon_stop_nrt_profile(str(output_dir).encode())
                print(f"ntff profile: {n} file(s) -> {output_dir}",
                      file=sys.stderr)

        mod = types.ModuleType("antenv.axon_hooks")
        _state = {"hook": _hook}
        mod.set_axon_ntff_profile_hook = lambda h: _state.__setitem__("hook", h)
        mod.get_axon_ntff_profile_hook = lambda: _state["hook"]
        sys.modules["antenv.axon_hooks"] = mod
        import antenv
        antenv.axon_hooks = mod
    except Exception:
        pass


_install_ntff_hook()

F32 = mybir.dt.float32
F32R = mybir.dt.float32r
BF16 = mybir.dt.bfloat16
FP8 = mybir.dt.float8e4
NP_FP8 = ml_dtypes.float8_e4m3
ACT_EXP = mybir.ActivationFunctionType.Exp
ACT_COPY = mybir.ActivationFunctionType.Copy
DOUBLE_ROW = mybir.MatmulPerfMode.DoubleRow

# Problem constants (hardcoded per contract).
HIDDEN = 1024
HEADS = 16
HEAD_DIM = 64
GAMMA = 0.5
B, S = 2, 2048
N_CORES = 8
GROUPS = N_CORES // B  # head groups per batch
HPC = HEADS // GROUPS  # heads per core
LAG = 4  # kt software-pipeline lag between exp and ctx matmul
WSCALE = 256.0
EXP_DESCALE = 1.0 / (WSCALE * WSCALE)


def emit_kernel(tc, aps, *, S_, C_, HPC_, QB):
    """Emit the per-core kernel. aps: dict of dram APs."""
    nc = tc.nc
    CT = C_ // 128          # contraction tiles over hidden
    CP = CT // 2            # contraction pairs for fp8 DoubleRow
    PAIRS = HPC_ // 2       # head pairs (128-channel chunks)
    D2 = HPC_ * HEAD_DIM
    NKT = S_ // 128         # key tiles
    NQB = S_ // QB          # query blocks
    PB = min(512, S_)       # projection free-block width
    NPB = S_ // PB
    KPB = PB // 128         # key tiles per nb block
    OB_W = min(512, C_)     # out-projection free-block width
    NOB = C_ // OB_W

    xh8_d = aps["xh8"]; xT_d = aps["xT"]
    wq_d = aps["wq"]; wk_d = aps["wk"]; wv_d = aps["wv"]; wo_d = aps["wo"]
    bq_d = aps["bq"]; bk_d = aps["bk"]; bv_d = aps["bv"]
    out_d = aps["out"]
    mask_d = aps.get("maskadd")

    from contextlib import ExitStack
    stack = ExitStack()
    consts = stack.enter_context(tc.tile_pool(name="consts", bufs=1))
    xpool = stack.enter_context(tc.tile_pool(name="xpool", bufs=1))
    projpool = stack.enter_context(tc.tile_pool(name="projpool", bufs=1))

    wo_sb = consts.tile([128, PAIRS, C_], F32R)
    xh8_sb = xpool.tile([128, CT, S_], FP8)

    # projections (fp32r, pre-scaled by 256; V in [keys, 1+dims] layout
    # with the 256-valued denominator column at position 0)
    qT_sb = projpool.tile([128, PAIRS, S_], F32R)
    kT_sb = projpool.tile([128, PAIRS, S_], F32R)
    v2_sb = projpool.tile([128, HPC_, NKT, HEAD_DIM + 1], F32R)

    with tc.tile_pool(name="xtpool", bufs=1) as xtpool, \
         tc.tile_pool(name="wstage", bufs=1) as wstage, \
         tc.tile_pool(name="wpool", bufs=1) as wpool, \
         tc.tile_pool(name="ph1psum", bufs=2, space="PSUM") as prj_ps, \
         tc.tile_pool(name="vpsum", bufs=2, space="PSUM") as v_ps:
        # x^T arrives as bf16 (host cast); per-nb tiles so the first
        # projection block starts after ~1MB of DMA instead of 4MB
        xTnb = [xtpool.tile([128, CT, PB], BF16, name=f"xT_{nb}")
                for nb in range(NPB)]
        # weights arrive as bf16 from the host: DMA straight in, no rounding
        wq_sb = wpool.tile([128, CT, D2], BF16)
        wk_sb = wpool.tile([128, CT, D2], BF16)
        wv_sb = wpool.tile([128, CT, D2], BF16)
        nc.sync.dma_start(out=wq_sb,
                          in_=wq_d.rearrange("(t p) m -> p t m", p=128))
        xT_r = xT_d.rearrange("(t p) m -> p t m", p=128)
        for nb in range(NPB):
            nc.sync.dma_start(out=xTnb[nb],
                              in_=xT_r[:, :, nb * PB:(nb + 1) * PB])
        for w_sb, w_d in ((wk_sb, wk_d), (wv_sb, wv_d)):
            nc.sync.dma_start(out=w_sb,
                              in_=w_d.rearrange("(t p) m -> p t m", p=128))
        bq_sb = wpool.tile([128, PAIRS, 1], F32)
        bk_sb = wpool.tile([128, PAIRS, 1], F32)
        for b_sb, b_d in ((bq_sb, bq_d), (bk_sb, bk_d)):
            nc.sync.dma_start(
                out=b_sb, in_=b_d.rearrange("(j p) one -> p j one", p=128))
        vbias_sb = wpool.tile([128, D2], F32)
        nc.sync.dma_start(out=vbias_sb, in_=bv_d.to_broadcast([128, D2]))
        for c in range(CT):
            nc.sync.dma_start(out=xh8_sb[:, c, :],
                              in_=xh8_d[c * 128:(c + 1) * 128, :])
        wos = wstage.tile([128, PAIRS, C_], F32, tag="ws", name="wos")
        nc.sync.dma_start(out=wos, in_=wo_d.rearrange("(j p) o -> p j o", p=128))
        nc.vector.tensor_copy(wo_sb, wos)

        for w_sb, b_sb, dest in (
            (wq_sb, bq_sb, qT_sb),
            (wk_sb, bk_sb, kT_sb),
        ):
            for nb in range(NPB):
                pss = [prj_ps.tile([128, PB], F32, tag=f"prj{j}",
                                   name=f"prj_{dest.tensor.name}_{nb}_{j}")
                       for j in range(PAIRS)]
                for c in range(CT):
                    for j in range(PAIRS):
                        nc.tensor.matmul(
                            pss[j],
                            w_sb[:, c, j * 128:(j + 1) * 128],
                            xTnb[nb][:, c, :],
                            start=(c == 0),
                            stop=(c == CT - 1),
                        )
                for j in range(PAIRS):
                    nc.vector.tensor_scalar_add(
                        dest[:, j, nb * PB:(nb + 1) * PB], pss[j], b_sb[:, j, :]
                    )

        # V computed directly with keys on partitions (lhsT = x^T chunk)
        for kt in range(NKT):
            nb, off = divmod(kt, PB // 128)
            off *= 128
            vp = v_ps.tile([128, D2], F32, tag="vp")
            for c in range(CT):
                nc.tensor.matmul(
                    vp,
                    xTnb[nb][:, c, off:off + 128],
                    wv_sb[:, c, :],
                    start=(c == 0),
                    stop=(c == CT - 1),
                )
            nc.vector.tensor_add(
                v2_sb[:, :, kt, 0:HEAD_DIM],
                vp.rearrange("p (h d) -> p h d", h=HPC_),
                vbias_sb.rearrange("p (h d) -> p h d", h=HPC_),
            )
        onescol = wstage.tile([128, HPC_, NKT, 1], F32)
        nc.vector.memset(onescol, WSCALE)
        nc.vector.tensor_copy(v2_sb[:, :, :, HEAD_DIM:HEAD_DIM + 1], onescol)

    # per-qb ctx tiles so the out-projection of early query blocks does not
    # serialize on the last block's divisions (whole-tile dependency)
    ctxT2_nq = [projpool.tile([128, PAIRS, QB], F32R, name=f"ctxT2_{nq}")
                for nq in range(NQB)]

    # --- main loop (phase 2) ---
    ptpool = stack.enter_context(tc.tile_pool(name="ptpool", bufs=11))
    spool = stack.enter_context(tc.tile_pool(name="spool", bufs=2))
    simsb = stack.enter_context(tc.tile_pool(name="simsb", bufs=2))
    smallpool = stack.enter_context(tc.tile_pool(name="smallpool", bufs=2))
    mpool = (stack.enter_context(tc.tile_pool(name="mpool", bufs=2))
             if mask_d is not None else None)

    with tc.tile_pool(name="simpsum", bufs=2, space="PSUM") as simp, \
         tc.tile_pool(name="scpsum", bufs=1, space="PSUM") as scp, \
         tc.tile_pool(name="ctxpsum", bufs=1, space="PSUM") as ctxp:

        def emit_ctx(ctx_ps, kt, pt_pairs):
            for j in range(PAIRS):
                for hi in range(2):
                    nc.tensor.matmul(
                        ctx_ps[2 * j + hi],
                        v2_sb[:, 2 * j + hi, kt, :],
                        pt_pairs[j][:, hi, :],
                        start=(kt == 0),
                        stop=(kt == NKT - 1),
                        skip_group_check=True,
                    )

        def emit_division_head(qb, ctx_ps, h):
            # short chain: single-op approx reciprocal of the denominator
            # row (DVE, PSUM read), GpSimd partition-0 broadcast, DVE mul.
            j, hi = divmod(h, 2)
            r0c = smallpool.tile([1, QB], F32, tag=f"r0c{h % 2}",
                                 name=f"r0c_{qb}_{h}")
            nc.vector.tensor_copy(r0c, ctx_ps[h][HEAD_DIM:HEAD_DIM + 1, :])
            r0 = smallpool.tile([1, QB], F32, tag=f"r0{h % 2}",
                                name=f"r0_{qb}_{h}")
            nc.vector.reciprocal_approx_fast(out=r0, in_=r0c)
            rb = smallpool.tile([HEAD_DIM, QB], F32, tag="rb",
                                name=f"rb_{qb}_{h}")
            nc.gpsimd.partition_broadcast(rb, r0, channels=HEAD_DIM)
            nc.vector.tensor_mul(
                ctxT2_nq[qb][hi * 64:hi * 64 + 64, j, :],
                ctx_ps[h][0:HEAD_DIM, :],
                rb,
            )

        def emit_division(qb, ctx_ps):
            for h in range(HPC_):
                emit_division_head(qb, ctx_ps, h)

        prev_div = None
        for qb in range(NQB):
            ctx_ps = [ctxp.tile([HEAD_DIM + 1, QB], F32, tag=f"ctx{h}",
                                name=f"ctx_{qb}_{h}")
                      for h in range(HPC_)]
            pending = []
            for kt in range(NKT):
                sp = simp.tile([128, QB], F32, tag="sim")
                for cp in range(CP):
                    nc.tensor.matmul(
                        sp,
                        xh8_sb[:, 2 * cp:2 * cp + 2, kt * 128:(kt + 1) * 128],
                        xh8_sb[:, 2 * cp:2 * cp + 2, qb * QB:(qb + 1) * QB],
                        start=(cp == 0),
                        stop=(cp == CP - 1),
                        perf_mode=DOUBLE_ROW,
                    )
                # HW allows only one PSUM operand per DVE op: stage sim in
                # SBUF on ACT (the DVE queue is the oversubscribed one)
                sim_t = simsb.tile([128, QB], F32, tag="simsb")
                nc.scalar.activation(out=sim_t, in_=sp, func=ACT_COPY)
                sim_in = sim_t.unsqueeze(1).to_broadcast([128, 2, QB])
                if mask_d is not None:
                    m_sb = mpool.tile([128, QB], F32, tag="msk")
                    nc.sync.dma_start(
                        out=m_sb,
                        in_=mask_d[kt * 128:(kt + 1) * 128, qb * QB:(qb + 1) * QB],
                    )
                pt_pairs = []
                for j in range(PAIRS):
                    sc_t = scp.tile([128, 2, QB], F32, tag="scp")
                    for hi in range(2):
                        pr = slice(hi * 64, hi * 64 + 64)
                        nc.tensor.matmul(
                            sc_t[:, hi, :],
                            kT_sb[pr, j, kt * 128:(kt + 1) * 128],
                            qT_sb[pr, j, qb * QB:(qb + 1) * QB],
                            start=True,
                            stop=True,
                        )
                    # interleave ctx matmuls (lagged) between the two score
                    # pairs so the PE has work while the DVE runs the sub
                    if j == 0 and pending and len(pending) > LAG:
                        k0, p0 = pending.pop(0)
                        emit_ctx(ctx_ps, k0, p0)
                    s_t = spool.tile([128, 2, QB], F32, tag="s")
                    nc.vector.tensor_sub(s_t, sc_t, sim_in)
                    if mask_d is not None:
                        nc.vector.tensor_sub(
                            s_t, s_t, m_sb.unsqueeze(1).to_broadcast([128, 2, QB]))
                    pt = ptpool.tile([128, 2, QB], F32R, tag="pt")
                    nc.scalar.activation(out=pt, in_=s_t, func=ACT_EXP,
                                         scale=EXP_DESCALE)
                    pt_pairs.append(pt)
                pending.append((kt, pt_pairs))
                # previous block's divisions, spread over the kt loop
                if prev_div is not None and kt >= 1 and (kt - 1) % 3 == 0:
                    h = (kt - 1) // 3
                    if h < HPC_:
                        emit_division_head(prev_div[0], prev_div[1], h)
                        if h == HPC_ - 1:
                            prev_div = None
            for k0, p0 in pending:
                emit_ctx(ctx_ps, k0, p0)
            if prev_div is not None:
                done = max(0, (NKT - 1 - 1) // 3 + 1) if NKT > 1 else 0
                for h in range(min(done, HPC_), HPC_):
                    emit_division_head(prev_div[0], prev_div[1], h)
                prev_div = None
            prev_div = (qb, ctx_ps)
        emit_division(*prev_div)

    # --- out-projection (phase 3) ---
    with tc.tile_pool(name="outpsum", bufs=4, space="PSUM") as outp, \
         tc.tile_pool(name="outstg", bufs=4) as outstg:
        QT_PER = QB // 128
        for qt in range(S_ // 128):
            for ob in range(NOB):
                op = outp.tile([128, OB_W], F32, tag="op")
                for j in range(PAIRS):
                    nc.tensor.matmul(
                        op,
                        ctxT2_nq[qt // QT_PER][
                            :, j, (qt % QT_PER) * 128:(qt % QT_PER + 1) * 128],
                        wo_sb[:, j, ob * OB_W:(ob + 1) * OB_W],
                        start=(j == 0),
                        stop=(j == PAIRS - 1),
                    )
                ostg = outstg.tile([128, OB_W], BF16, tag="ostg")
                if (qt + ob) % 2 == 0:
                    nc.scalar.activation(out=ostg, in_=op, func=ACT_COPY)
                else:
                    nc.vector.tensor_copy(ostg, op)
                nc.sync.dma_start(
                    out=out_d[qt * 128:(qt + 1) * 128, ob * OB_W:(ob + 1) * OB_W],
                    in_=ostg,
                )

    stack.close()


def build_nc(*, S_=S, C_=HIDDEN, HPC_=HPC, QB=512, with_mask=False,
             enable_asserts=False):
    nc = bacc.Bacc(
        "TRN2", target_bir_lowering=False, debug=False,
        enable_asserts=enable_asserts,
    )
    D2 = HPC_ * HEAD_DIM
    aps = {}
    aps["xT"] = nc.dram_tensor("xT", [C_, S_], BF16, kind="ExternalInput").ap()
    aps["xh8"] = nc.dram_tensor("xh8", [C_, S_], FP8, kind="ExternalInput").ap()
    for n in ("wq", "wk", "wv"):
        aps[n] = nc.dram_tensor(n, [C_, D2], BF16, kind="ExternalInput").ap()
    aps["wo"] = nc.dram_tensor("wo", [D2, C_], F32, kind="ExternalInput").ap()
    for n in ("bq", "bk"):
        aps[n] = nc.dram_tensor(n, [D2, 1], F32, kind="ExternalInput").ap()
    aps["bv"] = nc.dram_tensor("bv", [1, D2], F32, kind="ExternalInput").ap()
    if with_mask:
        aps["maskadd"] = nc.dram_tensor(
            "maskadd", [S_, S_], F32, kind="ExternalInput").ap()
    aps["out"] = nc.dram_tensor("out", [S_, C_], BF16, kind="ExternalOutput").ap()

    with tile.TileContext(nc) as tc:
        emit_kernel(tc, aps, S_=S_, C_=C_, HPC_=HPC_, QB=QB)
    nc.compile()
    return nc


def _q8(a):
    return a.astype(NP_FP8)


def host_prepare(x, attn_mask, Wq, bq, Wk, bk, Wv, bv, Wo, bo, *,
                 S_=S, C_=HIDDEN, HPC_=HPC, n_cores=N_CORES):
    """Build the per-core input maps. Returns (in_maps, with_mask)."""
    x = np.asarray(x, np.float32)
    B_ = x.shape[0]
    groups = n_cores // B_
    Wq = np.asarray(Wq, np.float32); Wk = np.asarray(Wk, np.float32)
    Wv = np.asarray(Wv, np.float32); Wo = np.asarray(Wo, np.float32)
    bq = np.asarray(bq, np.float32); bk = np.asarray(bk, np.float32)
    bv = np.asarray(bv, np.float32)

    inv_sqrt_d = 1.0 / math.sqrt(HEAD_DIM)
    # weights carry the 256x matched scale so scores_psum = 65536 * s;
    # cast to bf16 on host (device DMAs them straight into SBUF)
    WqT = np.ascontiguousarray(
        (Wq.T * (inv_sqrt_d * WSCALE)).astype(ml_dtypes.bfloat16))
    WkT = np.ascontiguousarray((Wk.T * WSCALE).astype(ml_dtypes.bfloat16))
    WvT = np.ascontiguousarray((Wv.T * WSCALE).astype(ml_dtypes.bfloat16))
    WoT = np.ascontiguousarray(Wo.T)                 # [C(c), C(o)]
    bq_s = bq * (inv_sqrt_d * WSCALE)
    bk_s = bk * WSCALE
    bv_s = bv * WSCALE

    mask = np.asarray(attn_mask)
    with_mask = bool(mask.any())
    maskadd = None
    if with_mask:
        # s_t -= maskadd; masked positions get -1e20/65536 pre-exp -> 0
        maskadd = np.where(mask, np.float32(1e20), np.float32(0.0)).astype(np.float32)
        maskadd = np.ascontiguousarray(maskadd.T)  # [k, q]

    in_maps = []
    per_b = {}
    for core in range(n_cores):
        b, g = divmod(core, groups)
        if b not in per_b:
            xb = x[b]                               # [S, C]
            xT = np.ascontiguousarray(xb.T)         # [C, S]
            norms = np.linalg.norm(xb, axis=1)      # [S]
            scale = (math.sqrt(GAMMA) * WSCALE /
                     np.maximum(norms, 1e-12)).astype(np.float32)
            xh8 = _q8(xT * scale[None, :])
            xT16 = np.ascontiguousarray(xT.astype(ml_dtypes.bfloat16))
            per_b[b] = (xT16, np.ascontiguousarray(xh8))
        xT, xh8 = per_b[b]
        ch = slice(g * HPC_ * HEAD_DIM, (g + 1) * HPC_ * HEAD_DIM)
        m = {
            "xT": xT, "xh8": xh8,
            "wq": np.ascontiguousarray(WqT[:, ch]),
            "wk": np.ascontiguousarray(WkT[:, ch]),
            "wv": np.ascontiguousarray(WvT[:, ch]),
            "wo": np.ascontiguousarray(WoT[ch, :]),
            "bq": np.ascontiguousarray(bq_s[ch]).reshape(-1, 1),
            "bk": np.ascontiguousarray(bk_s[ch]).reshape(-1, 1),
            "bv": np.ascontiguousarray(bv_s[ch]).reshape(1, -1),
        }
        if with_mask:
            m["maskadd"] = maskadd
        in_maps.append(m)
    return in_maps, with_mask


_NC_CACHE = {}


def _get_nc(with_mask):
    key = with_mask
    if key not in _NC_CACHE:
        _NC_CACHE[key] = build_nc(with_mask=with_mask)
    return _NC_CACHE[key]


LAST_RESULTS = None


def kernel(**inputs):
    global LAST_RESULTS
    in_maps, with_mask = host_prepare(
        inputs["x"], inputs["attn_mask"],
        inputs["Wq"], inputs["bq"], inputs["Wk"], inputs["bk"],
        inputs["Wv"], inputs["bv"], inputs["Wo"], inputs["bo"],
    )
    nc = _get_nc(with_mask)
    res = run_bass_kernel_spmd(nc, in_maps, core_ids=list(range(N_CORES)))
    LAST_RESULTS = res
    bo = np.asarray(inputs["bo"], np.float32)
    out = np.zeros((B, S, HIDDEN), np.float32)
    groups = N_CORES // B
    for core in range(N_CORES):
        b = core // groups
        out[b] += res.results[core]["out"].astype(np.float32)
    out += bo[None, None, :]
    return out


# revision 62
# speedup vs baseline: 1.0740x; 1.0057x over previous
"""DiversityAttention on 8 TRN2 NeuronCores (Bass/Tile).

Sharding: data-parallel over batch (B=2) x tensor-parallel over heads
(16 heads -> 4 groups of 4). core = (b, g), b = core // 4, g = core % 4.
Each core computes full attention for its 4 heads over its batch and a
partial out-projection [S, HIDDEN]; the host sums the 4 partials per
batch and adds bo.

All big matmuls run as fp8e4 DoubleRow (0.25x the fp32r cost) where
precision allows; q/k/v projections use a 3-term fp8 residual
decomposition (W8@x8 + W8@xr8 + Wr8@x8 ~ W@x to ~0.1%).

Scale conventions (host-side):
  wq8/wqr8 = fp8(256 * Wq^T / sqrt(dh)), bq' = 256*bq/sqrt(dh)
  wk8/wkr8 = fp8(256 * Wk^T),            bk' = 256*bk
  wv8/wvr8 = fp8(256 * Wv^T),            bv' = 256*bv
  x8/xr8   = fp8(x^T) + fp8 residual
  xh8      = fp8(sqrt(gamma) * 256 * x^T / max(||x||, eps))
so on device:
  qT_sb = 256*q, kT_sb = 256*k   -> scores_psum = 65536 * s
  sim_psum = 65536 * gamma * sim -> s_t = 65536*(s - gamma*sim)
  P = exp(s_t / 65536)  (activation scale)
  v2 = [256 | 256*v] per head (ones col at 0 for the denominator row);
  ctx_psum row 0 = 256*sum(P), rows 1..64 = 256*sum(v P): ratio exact.
Division runs fully on GpSimd: partition_broadcast(denom row 0) ->
reciprocal -> multiply, keeping the DVE queue free for the score-sim
subtractions.
"""

import math
import os
import sys

import numpy as np

for _p in ("/opt/trn_rl_repo",):
    if _p not in sys.path and os.path.isdir(_p):
        sys.path.insert(0, _p)

os.environ.setdefault("MYCRO_LOCAL_CACHE", "1")

import ml_dtypes

import concourse.bass as bass
import concourse.tile as tile
from concourse import bacc, mybir
from concourse.bass_utils import run_bass_kernel_spmd


def _install_ntff_hook():
    """Provide antenv.axon_hooks (NTFF profiling registry) if the image
    lacks it, mirroring trn_agent_boot's ctypes hook. No-op on failure."""
    try:
        import antenv.axon_hooks  # noqa: F401
        return
    except ImportError:
        pass
    try:
        import contextlib
        import ctypes
        import types

        so_path = "/opt/axon/libaxon_pjrt.so"
        if not os.path.exists(so_path):
            return
        lib = ctypes.CDLL(so_path)
        if not hasattr(lib, "axon_start_nrt_profile"):
            return
        lib.axon_start_nrt_profile.argtypes = [
            ctypes.POINTER(ctypes.c_int64), ctypes.c_size_t]
        lib.axon_start_nrt_profile.restype = ctypes.c_int64
        lib.axon_stop_nrt_profile.argtypes = [ctypes.c_char_p]
        lib.axon_stop_nrt_profile.restype = ctypes.c_int64

        @contextlib.contextmanager
        def _hook(output_dir, device_ids):
            import jax
            jax.devices()
            if device_ids:
                ids = (ctypes.c_int64 * len(device_ids))(*device_ids)
                rc = lib.axon_start_nrt_profile(ids, len(device_ids))
            else:
                rc = lib.axon_start_nrt_profile(None, 0)
            if rc != 0:
                raise RuntimeError(f"axon_start_nrt_profile rc={rc}")
            try:
                yield
            finally:
                n = lib.axon_stop_nrt_profile(str(output_dir).encode())
                print(f"ntff profile: {n} file(s) -> {output_dir}",
                      file=sys.stderr)

        mod = types.ModuleType("antenv.axon_hooks")
        _state = {"hook": _hook}
        mod.set_axon_ntff_profile_hook = lambda h: _state.__setitem__("hook", h)
        mod.get_axon_ntff_profile_hook = lambda: _state["hook"]
        sys.modules["antenv.axon_hooks"] = mod
        import antenv
        antenv.axon_hooks = mod
    except Exception:
        pass


_install_ntff_hook()

F32 = mybir.dt.float32
F32R = mybir.dt.float32r
BF16 = mybir.dt.bfloat16
FP8 = mybir.dt.float8e4
NP_FP8 = ml_dtypes.float8_e4m3
ACT_EXP = mybir.ActivationFunctionType.Exp
ACT_COPY = mybir.ActivationFunctionType.Copy
DOUBLE_ROW = mybir.MatmulPerfMode.DoubleRow

# Problem constants (hardcoded per contract).
HIDDEN = 1024
HEADS = 16
HEAD_DIM = 64
GAMMA = 0.5
B, S = 2, 2048
N_CORES = 8
GROUPS = N_CORES // B  # head groups per batch
HPC = HEADS // GROUPS  # heads per core
LAG = 5  # kt software-pipeline lag between exp and ctx matmul
WSCALE = 256.0
EXP_DESCALE = 1.0 / (WSCALE * WSCALE)


def emit_kernel(tc, aps, *, S_, C_, HPC_, QB):
    """Emit the per-core kernel. aps: dict of dram APs."""
    nc = tc.nc
    CT = C_ // 128          # contraction tiles over hidden
    CP = CT // 2            # contraction pairs for fp8 DoubleRow
    PAIRS = HPC_ // 2       # head pairs (128-channel chunks)
    D2 = HPC_ * HEAD_DIM
    NKT = S_ // 128         # key tiles
    NQB = S_ // QB          # query blocks
    PB = min(512, S_)       # projection free-block width
    NPB = S_ // PB
    KPB = PB // 128         # key tiles per nb block
    OB_W = min(512, C_)     # out-projection free-block width
    NOB = C_ // OB_W

    xh8_d = aps["xh8"]; xT_d = aps["xT"]
    wq_d = aps["wq"]; wk_d = aps["wk"]; wv_d = aps["wv"]; wo_d = aps["wo"]
    bq_d = aps["bq"]; bk_d = aps["bk"]; bv_d = aps["bv"]
    out_d = aps["out"]
    mask_d = aps.get("maskadd")

    from contextlib import ExitStack
    stack = ExitStack()
    consts = stack.enter_context(tc.tile_pool(name="consts", bufs=1))
    xpool = stack.enter_context(tc.tile_pool(name="xpool", bufs=1))
    projpool = stack.enter_context(tc.tile_pool(name="projpool", bufs=1))

    wo_sb = consts.tile([128, PAIRS, C_], F32R)
    xh8_sb = xpool.tile([128, CT, S_], FP8)

    # projections (fp32r, pre-scaled by 256; V in [keys, 1+dims] layout
    # with the 256-valued denominator column at position 0)
    qT_sb = projpool.tile([128, PAIRS, S_], F32R)
    kT_sb = projpool.tile([128, PAIRS, S_], F32R)
    v2_sb = projpool.tile([128, HPC_, NKT, HEAD_DIM + 1], F32R)

    with tc.tile_pool(name="xtpool", bufs=1) as xtpool, \
         tc.tile_pool(name="wstage", bufs=1) as wstage, \
         tc.tile_pool(name="wpool", bufs=1) as wpool, \
         tc.tile_pool(name="ph1psum", bufs=2, space="PSUM") as prj_ps, \
         tc.tile_pool(name="vpsum", bufs=2, space="PSUM") as v_ps:
        # x^T arrives as bf16 (host cast); per-nb tiles so the first
        # projection block starts after ~1MB of DMA instead of 4MB
        xTnb = [xtpool.tile([128, CT, PB], BF16, name=f"xT_{nb}")
                for nb in range(NPB)]
        # weights arrive as bf16 from the host: DMA straight in, no rounding
        wq_sb = wpool.tile([128, CT, D2], BF16)
        wk_sb = wpool.tile([128, CT, D2], BF16)
        wv_sb = wpool.tile([128, CT, D2], BF16)
        nc.sync.dma_start(out=wq_sb,
                          in_=wq_d.rearrange("(t p) m -> p t m", p=128))
        xT_r = xT_d.rearrange("(t p) m -> p t m", p=128)
        for nb in range(NPB):
            nc.sync.dma_start(out=xTnb[nb],
                              in_=xT_r[:, :, nb * PB:(nb + 1) * PB])
        for w_sb, w_d in ((wk_sb, wk_d), (wv_sb, wv_d)):
            nc.sync.dma_start(out=w_sb,
                              in_=w_d.rearrange("(t p) m -> p t m", p=128))
        bq_sb = wpool.tile([128, PAIRS, 1], F32)
        bk_sb = wpool.tile([128, PAIRS, 1], F32)
        for b_sb, b_d in ((bq_sb, bq_d), (bk_sb, bk_d)):
            nc.sync.dma_start(
                out=b_sb, in_=b_d.rearrange("(j p) one -> p j one", p=128))
        vbias_sb = wpool.tile([128, D2], F32)
        nc.sync.dma_start(out=vbias_sb, in_=bv_d.to_broadcast([128, D2]))
        for c in range(CT):
            nc.sync.dma_start(out=xh8_sb[:, c, :],
                              in_=xh8_d[c * 128:(c + 1) * 128, :])
        wos = wstage.tile([128, PAIRS, C_], F32, tag="ws", name="wos")
        nc.sync.dma_start(out=wos, in_=wo_d.rearrange("(j p) o -> p j o", p=128))
        nc.vector.tensor_copy(wo_sb, wos)

        for w_sb, b_sb, dest in (
            (wq_sb, bq_sb, qT_sb),
            (wk_sb, bk_sb, kT_sb),
        ):
            for nb in range(NPB):
                pss = [prj_ps.tile([128, PB], F32, tag=f"prj{j}",
                                   name=f"prj_{dest.tensor.name}_{nb}_{j}")
                       for j in range(PAIRS)]
                for c in range(CT):
                    for j in range(PAIRS):
                        nc.tensor.matmul(
                            pss[j],
                            w_sb[:, c, j * 128:(j + 1) * 128],
                            xTnb[nb][:, c, :],
                            start=(c == 0),
                            stop=(c == CT - 1),
                        )
                for j in range(PAIRS):
                    nc.vector.tensor_scalar_add(
                        dest[:, j, nb * PB:(nb + 1) * PB], pss[j], b_sb[:, j, :]
                    )

        # V computed directly with keys on partitions (lhsT = x^T chunk)
        for kt in range(NKT):
            nb, off = divmod(kt, PB // 128)
            off *= 128
            vp = v_ps.tile([128, D2], F32, tag="vp")
            for c in range(CT):
                nc.tensor.matmul(
                    vp,
                    xTnb[nb][:, c, off:off + 128],
                    wv_sb[:, c, :],
                    start=(c == 0),
                    stop=(c == CT - 1),
                )
            nc.vector.tensor_add(
                v2_sb[:, :, kt, 0:HEAD_DIM],
                vp.rearrange("p (h d) -> p h d", h=HPC_),
                vbias_sb.rearrange("p (h d) -> p h d", h=HPC_),
            )
        onescol = wstage.tile([128, HPC_, NKT, 1], F32)
        nc.vector.memset(onescol, WSCALE)
        nc.vector.tensor_copy(v2_sb[:, :, :, HEAD_DIM:HEAD_DIM + 1], onescol)

    # per-qb ctx tiles so the out-projection of early query blocks does not
    # serialize on the last block's divisions (whole-tile dependency)
    ctxT2_nq = [projpool.tile([128, PAIRS, QB], F32R, name=f"ctxT2_{nq}")
                for nq in range(NQB)]

    # --- main loop (phase 2) ---
    ptpool = stack.enter_context(tc.tile_pool(name="ptpool", bufs=13))
    spool = stack.enter_context(tc.tile_pool(name="spool", bufs=2))
    simsb = stack.enter_context(tc.tile_pool(name="simsb", bufs=2))
    smallpool = stack.enter_context(tc.tile_pool(name="smallpool", bufs=2))
    mpool = (stack.enter_context(tc.tile_pool(name="mpool", bufs=2))
             if mask_d is not None else None)

    with tc.tile_pool(name="simpsum", bufs=2, space="PSUM") as simp, \
         tc.tile_pool(name="scpsum", bufs=1, space="PSUM") as scp, \
         tc.tile_pool(name="ctxpsum", bufs=1, space="PSUM") as ctxp:

        def emit_ctx(ctx_ps, kt, pt_pairs):
            for j in range(PAIRS):
                for hi in range(2):
                    nc.tensor.matmul(
                        ctx_ps[2 * j + hi],
                        v2_sb[:, 2 * j + hi, kt, :],
                        pt_pairs[j][:, hi, :],
                        start=(kt == 0),
                        stop=(kt == NKT - 1),
                        skip_group_check=True,
                    )

        def emit_division_head(qb, ctx_ps, h):
            # short chain: single-op approx reciprocal of the denominator
            # row (DVE, PSUM read), GpSimd partition-0 broadcast, DVE mul.
            j, hi = divmod(h, 2)
            r0c = smallpool.tile([1, QB], F32, tag=f"r0c{h % 2}",
                                 name=f"r0c_{qb}_{h}")
            nc.vector.tensor_copy(r0c, ctx_ps[h][HEAD_DIM:HEAD_DIM + 1, :])
            r0 = smallpool.tile([1, QB], F32, tag=f"r0{h % 2}",
                                name=f"r0_{qb}_{h}")
            nc.vector.reciprocal_approx_fast(out=r0, in_=r0c)
            rb = smallpool.tile([HEAD_DIM, QB], F32, tag="rb",
                                name=f"rb_{qb}_{h}")
            nc.gpsimd.partition_broadcast(rb, r0, channels=HEAD_DIM)
            nc.vector.tensor_mul(
                ctxT2_nq[qb][hi * 64:hi * 64 + 64, j, :],
                ctx_ps[h][0:HEAD_DIM, :],
                rb,
            )

        def emit_division(qb, ctx_ps):
            for h in range(HPC_):
                emit_division_head(qb, ctx_ps, h)

        prev_div = None
        for qb in range(NQB):
            ctx_ps = [ctxp.tile([HEAD_DIM + 1, QB], F32, tag=f"ctx{h}",
                                name=f"ctx_{qb}_{h}")
                      for h in range(HPC_)]
            pending = []
            for kt in range(NKT):
                sp = simp.tile([128, QB], F32, tag="sim")
                for cp in range(CP):
                    nc.tensor.matmul(
                        sp,
                        xh8_sb[:, 2 * cp:2 * cp + 2, kt * 128:(kt + 1) * 128],
                        xh8_sb[:, 2 * cp:2 * cp + 2, qb * QB:(qb + 1) * QB],
                        start=(cp == 0),
                        stop=(cp == CP - 1),
                        perf_mode=DOUBLE_ROW,
                    )
                # HW allows only one PSUM operand per DVE op: stage sim in
                # SBUF on ACT (the DVE queue is the oversubscribed one)
                sim_t = simsb.tile([128, QB], F32, tag="simsb")
                nc.scalar.activation(out=sim_t, in_=sp, func=ACT_COPY)
                sim_in = sim_t.unsqueeze(1).to_broadcast([128, 2, QB])
                if mask_d is not None:
                    m_sb = mpool.tile([128, QB], F32, tag="msk")
                    nc.sync.dma_start(
                        out=m_sb,
                        in_=mask_d[kt * 128:(kt + 1) * 128, qb * QB:(qb + 1) * QB],
                    )
                pt_pairs = []
                for j in range(PAIRS):
                    sc_t = scp.tile([128, 2, QB], F32, tag="scp")
                    for hi in range(2):
                        pr = slice(hi * 64, hi * 64 + 64)
                        nc.tensor.matmul(
                            sc_t[:, hi, :],
                            kT_sb[pr, j, kt * 128:(kt + 1) * 128],
                            qT_sb[pr, j, qb * QB:(qb + 1) * QB],
                            start=True,
                            stop=True,
                        )
                    # interleave ctx matmuls (lagged) between the two score
                    # pairs so the PE has work while the DVE runs the sub
                    if j == 0 and pending and len(pending) > LAG:
                        k0, p0 = pending.pop(0)
                        emit_ctx(ctx_ps, k0, p0)
                    s_t = spool.tile([128, 2, QB], F32, tag="s")
                    nc.vector.tensor_sub(s_t, sc_t, sim_in)
                    if mask_d is not None:
                        nc.vector.tensor_sub(
                            s_t, s_t, m_sb.unsqueeze(1).to_broadcast([128, 2, QB]))
                    pt = ptpool.tile([128, 2, QB], F32R, tag="pt")
                    nc.scalar.activation(out=pt, in_=s_t, func=ACT_EXP,
                                         scale=EXP_DESCALE)
                    pt_pairs.append(pt)
                pending.append((kt, pt_pairs))
                # previous block's divisions, spread over the kt loop
                if prev_div is not None and kt % 3 == 0:
                    h = kt // 3
                    if h < HPC_:
                        emit_division_head(prev_div[0], prev_div[1], h)
                        if h == HPC_ - 1:
                            prev_div = None
            for k0, p0 in pending:
                emit_ctx(ctx_ps, k0, p0)
            if prev_div is not None:
                done = (NKT - 1) // 3 + 1
                for h in range(min(done, HPC_), HPC_):
                    emit_division_head(prev_div[0], prev_div[1], h)
                prev_div = None
            prev_div = (qb, ctx_ps)
        emit_division(*prev_div)

    # --- out-projection (phase 3) ---
    with tc.tile_pool(name="outpsum", bufs=4, space="PSUM") as outp, \
         tc.tile_pool(name="outstg", bufs=4) as outstg:
        QT_PER = QB // 128
        for qt in range(S_ // 128):
            for ob in range(NOB):
                op = outp.tile([128, OB_W], F32, tag="op")
                for j in range(PAIRS):
                    nc.tensor.matmul(
                        op,
                        ctxT2_nq[qt // QT_PER][
                            :, j, (qt % QT_PER) * 128:(qt % QT_PER + 1) * 128],
                        wo_sb[:, j, ob * OB_W:(ob + 1) * OB_W],
                        start=(j == 0),
                        stop=(j == PAIRS - 1),
                    )
                ostg = outstg.tile([128, OB_W], BF16, tag="ostg")
                if (qt + ob) % 2 == 0:
                    nc.scalar.activation(out=ostg, in_=op, func=ACT_COPY)
                else:
                    nc.vector.tensor_copy(ostg, op)
                nc.sync.dma_start(
                    out=out_d[qt * 128:(qt + 1) * 128, ob * OB_W:(ob + 1) * OB_W],
                    in_=ostg,
                )

    stack.close()


def build_nc(*, S_=S, C_=HIDDEN, HPC_=HPC, QB=512, with_mask=False,
             enable_asserts=False):
    nc = bacc.Bacc(
        "TRN2", target_bir_lowering=False, debug=False,
        enable_asserts=enable_asserts,
    )
    D2 = HPC_ * HEAD_DIM
    aps = {}
    aps["xT"] = nc.dram_tensor("xT", [C_, S_], BF16, kind="ExternalInput").ap()
    aps["xh8"] = nc.dram_tensor("xh8", [C_, S_], FP8, kind="ExternalInput").ap()
    for n in ("wq", "wk", "wv"):
        aps[n] = nc.dram_tensor(n, [C_, D2], BF16, kind="ExternalInput").ap()
    aps["wo"] = nc.dram_tensor("wo", [D2, C_], F32, kind="ExternalInput").ap()
    for n in ("bq", "bk"):
        aps[n] = nc.dram_tensor(n, [D2, 1], F32, kind="ExternalInput").ap()
    aps["bv"] = nc.dram_tensor("bv", [1, D2], F32, kind="ExternalInput").ap()
    if with_mask:
        aps["maskadd"] = nc.dram_tensor(
            "maskadd", [S_, S_], F32, kind="ExternalInput").ap()
    aps["out"] = nc.dram_tensor("out", [S_, C_], BF16, kind="ExternalOutput").ap()

    with tile.TileContext(nc) as tc:
        emit_kernel(tc, aps, S_=S_, C_=C_, HPC_=HPC_, QB=QB)
    nc.compile()
    return nc


def _q8(a):
    return a.astype(NP_FP8)


def host_prepare(x, attn_mask, Wq, bq, Wk, bk, Wv, bv, Wo, bo, *,
                 S_=S, C_=HIDDEN, HPC_=HPC, n_cores=N_CORES):
    """Build the per-core input maps. Returns (in_maps, with_mask)."""
    x = np.asarray(x, np.float32)
    B_ = x.shape[0]
    groups = n_cores // B_
    Wq = np.asarray(Wq, np.float32); Wk = np.asarray(Wk, np.float32)
    Wv = np.asarray(Wv, np.float32); Wo = np.asarray(Wo, np.float32)
    bq = np.asarray(bq, np.float32); bk = np.asarray(bk, np.float32)
    bv = np.asarray(bv, np.float32)

    inv_sqrt_d = 1.0 / math.sqrt(HEAD_DIM)
    # weights carry the 256x matched scale so scores_psum = 65536 * s;
    # cast to bf16 on host (device DMAs them straight into SBUF)
    WqT = np.ascontiguousarray(
        (Wq.T * (inv_sqrt_d * WSCALE)).astype(ml_dtypes.bfloat16))
    WkT = np.ascontiguousarray((Wk.T * WSCALE).astype(ml_dtypes.bfloat16))
    WvT = np.ascontiguousarray((Wv.T * WSCALE).astype(ml_dtypes.bfloat16))
    WoT = np.ascontiguousarray(Wo.T)                 # [C(c), C(o)]
    bq_s = bq * (inv_sqrt_d * WSCALE)
    bk_s = bk * WSCALE
    bv_s = bv * WSCALE

    mask = np.asarray(attn_mask)
    with_mask = bool(mask.any())
    maskadd = None
    if with_mask:
        # s_t -= maskadd; masked positions get -1e20/65536 pre-exp -> 0
        maskadd = np.where(mask, np.float32(1e20), np.float32(0.0)).astype(np.float32)
        maskadd = np.ascontiguousarray(maskadd.T)  # [k, q]

    in_maps = []
    per_b = {}
    for core in range(n_cores):
        b, g = divmod(core, groups)
        if b not in per_b:
            xb = x[b]                               # [S, C]
            xT = np.ascontiguousarray(xb.T)         # [C, S]
            norms = np.linalg.norm(xb, axis=1)      # [S]
            scale = (math.sqrt(GAMMA) * WSCALE /
                     np.maximum(norms, 1e-12)).astype(np.float32)
            xh8 = _q8(xT * scale[None, :])
            xT16 = np.ascontiguousarray(xT.astype(ml_dtypes.bfloat16))
            per_b[b] = (xT16, np.ascontiguousarray(xh8))
        xT, xh8 = per_b[b]
        ch = slice(g * HPC_ * HEAD_DIM, (g + 1) * HPC_ * HEAD_DIM)
        m = {
            "xT": xT, "xh8": xh8,
            "wq": np.ascontiguousarray(WqT[:, ch]),
            "wk": np.ascontiguousarray(WkT[:, ch]),
            "wv": np.ascontiguousarray(WvT[:, ch]),
            "wo": np.ascontiguousarray(WoT[ch, :]),
            "bq": np.ascontiguousarray(bq_s[ch]).reshape(-1, 1),
            "bk": np.ascontiguousarray(bk_s[ch]).reshape(-1, 1),
            "bv": np.ascontiguousarray(bv_s[ch]).reshape(1, -1),
        }
        if with_mask:
            m["maskadd"] = maskadd
        in_maps.append(m)
    return in_maps, with_mask


_NC_CACHE = {}


def _get_nc(with_mask):
    key = with_mask
    if key not in _NC_CACHE:
        _NC_CACHE[key] = build_nc(with_mask=with_mask)
    return _NC_CACHE[key]


LAST_RESULTS = None


def kernel(**inputs):
    global LAST_RESULTS
    in_maps, with_mask = host_prepare(
        inputs["x"], inputs["attn_mask"],
        inputs["Wq"], inputs["bq"], inputs["Wk"], inputs["bk"],
        inputs["Wv"], inputs["bv"], inputs["Wo"], inputs["bo"],
    )
    nc = _get_nc(with_mask)
    res = run_bass_kernel_spmd(nc, in_maps, core_ids=list(range(N_CORES)))
    LAST_RESULTS = res
    bo = np.asarray(inputs["bo"], np.float32)
    out = np.zeros((B, S, HIDDEN), np.float32)
    groups = N_CORES // B
    for core in range(N_CORES):
        b = core // groups
        out[b] += res.results[core]["out"].astype(np.float32)
    out += bo[None, None, :]
    return out


# revision 64
# speedup vs baseline: 1.0740x; 1.0000x over previous
"""DiversityAttention on 8 TRN2 NeuronCores (Bass/Tile).

Sharding: data-parallel over batch (B=2) x tensor-parallel over heads
(16 heads -> 4 groups of 4). core = (b, g), b = core // 4, g = core % 4.
Each core computes full attention for its 4 heads over its batch and a
partial out-projection [S, HIDDEN]; the host sums the 4 partials per
batch and adds bo.

All big matmuls run as fp8e4 DoubleRow (0.25x the fp32r cost) where
precision allows; q/k/v projections use a 3-term fp8 residual
decomposition (W8@x8 + W8@xr8 + Wr8@x8 ~ W@x to ~0.1%).

Scale conventions (host-side):
  wq8/wqr8 = fp8(256 * Wq^T / sqrt(dh)), bq' = 256*bq/sqrt(dh)
  wk8/wkr8 = fp8(256 * Wk^T),            bk' = 256*bk
  wv8/wvr8 = fp8(256 * Wv^T),            bv' = 256*bv
  x8/xr8   = fp8(x^T) + fp8 residual
  xh8      = fp8(sqrt(gamma) * 256 * x^T / max(||x||, eps))
so on device:
  qT_sb = 256*q, kT_sb = 256*k   -> scores_psum = 65536 * s
  sim_psum = 65536 * gamma * sim -> s_t = 65536*(s - gamma*sim)
  P = exp(s_t / 65536)  (activation scale)
  v2 = [256 | 256*v] per head (ones col at 0 for the denominator row);
  ctx_psum row 0 = 256*sum(P), rows 1..64 = 256*sum(v P): ratio exact.
Division runs fully on GpSimd: partition_broadcast(denom row 0) ->
reciprocal -> multiply, keeping the DVE queue free for the score-sim
subtractions.
"""

import math
import os
import sys

import numpy as np

for _p in ("/opt/trn_rl_repo",):
    if _p not in sys.path and os.path.isdir(_p):
        sys.path.insert(0, _p)

os.environ.setdefault("MYCRO_LOCAL_CACHE", "1")

import ml_dtypes

import concourse.bass as bass
import concourse.tile as tile
from concourse import bacc, mybir
from concourse.bass_utils import run_bass_kernel_spmd


def _install_ntff_hook():
    """Provide antenv.axon_hooks (NTFF profiling registry) if the image
    lacks it, mirroring trn_agent_boot's ctypes hook. No-op on failure."""
    try:
        import antenv.axon_hooks  # noqa: F401
        return
    except ImportError:
        pass
    try:
        import contextlib
        import ctypes
        import types

        so_path = "/opt/axon/libaxon_pjrt.so"
        if not os.path.exists(so_path):
            return
        lib = ctypes.CDLL(so_path)
        if not hasattr(lib, "axon_start_nrt_profile"):
            return
        lib.axon_start_nrt_profile.argtypes = [
            ctypes.POINTER(ctypes.c_int64), ctypes.c_size_t]
        lib.axon_start_nrt_profile.restype = ctypes.c_int64
        lib.axon_stop_nrt_profile.argtypes = [ctypes.c_char_p]
        lib.axon_stop_nrt_profile.restype = ctypes.c_int64

        @contextlib.contextmanager
        def _hook(output_dir, device_ids):
            import jax
            jax.devices()
            if device_ids:
                ids = (ctypes.c_int64 * len(device_ids))(*device_ids)
                rc = lib.axon_start_nrt_profile(ids, len(device_ids))
            else:
                rc = lib.axon_start_nrt_profile(None, 0)
            if rc != 0:
                raise RuntimeError(f"axon_start_nrt_profile rc={rc}")
            try:
                yield
            finally:
                n = lib.axon_stop_nrt_profile(str(output_dir).encode())
                print(f"ntff profile: {n} file(s) -> {output_dir}",
                      file=sys.stderr)

        mod = types.ModuleType("antenv.axon_hooks")
        _state = {"hook": _hook}
        mod.set_axon_ntff_profile_hook = lambda h: _state.__setitem__("hook", h)
        mod.get_axon_ntff_profile_hook = lambda: _state["hook"]
        sys.modules["antenv.axon_hooks"] = mod
        import antenv
        antenv.axon_hooks = mod
    except Exception:
        pass


_install_ntff_hook()

F32 = mybir.dt.float32
F32R = mybir.dt.float32r
BF16 = mybir.dt.bfloat16
FP8 = mybir.dt.float8e4
NP_FP8 = ml_dtypes.float8_e4m3
ACT_EXP = mybir.ActivationFunctionType.Exp
ACT_COPY = mybir.ActivationFunctionType.Copy
DOUBLE_ROW = mybir.MatmulPerfMode.DoubleRow

# Problem constants (hardcoded per contract).
HIDDEN = 1024
HEADS = 16
HEAD_DIM = 64
GAMMA = 0.5
B, S = 2, 2048
N_CORES = 8
GROUPS = N_CORES // B  # head groups per batch
HPC = HEADS // GROUPS  # heads per core
LAG = 5  # kt software-pipeline lag between exp and ctx matmul
WSCALE = 256.0
EXP_DESCALE = 1.0 / (WSCALE * WSCALE)


def emit_kernel(tc, aps, *, S_, C_, HPC_, QB):
    """Emit the per-core kernel. aps: dict of dram APs."""
    nc = tc.nc
    CT = C_ // 128          # contraction tiles over hidden
    CP = CT // 2            # contraction pairs for fp8 DoubleRow
    PAIRS = HPC_ // 2       # head pairs (128-channel chunks)
    D2 = HPC_ * HEAD_DIM
    NKT = S_ // 128         # key tiles
    NQB = S_ // QB          # query blocks
    PB = min(512, S_)       # projection free-block width
    NPB = S_ // PB
    KPB = PB // 128         # key tiles per nb block
    OB_W = min(512, C_)     # out-projection free-block width
    NOB = C_ // OB_W

    xh8_d = aps["xh8"]; xT_d = aps["xT"]
    wq_d = aps["wq"]; wk_d = aps["wk"]; wv_d = aps["wv"]; wo_d = aps["wo"]
    bq_d = aps["bq"]; bk_d = aps["bk"]; bv_d = aps["bv"]
    out_d = aps["out"]
    mask_d = aps.get("maskadd")

    from contextlib import ExitStack
    stack = ExitStack()
    consts = stack.enter_context(tc.tile_pool(name="consts", bufs=1))
    xpool = stack.enter_context(tc.tile_pool(name="xpool", bufs=1))
    projpool = stack.enter_context(tc.tile_pool(name="projpool", bufs=1))

    wo_sb = consts.tile([128, PAIRS, C_], F32R)
    xh8_sb = xpool.tile([128, CT, S_], FP8)

    # projections (fp32r, pre-scaled by 256; V in [keys, 1+dims] layout
    # with the 256-valued denominator column at position 0)
    qT_sb = projpool.tile([128, PAIRS, S_], F32R)
    kT_sb = projpool.tile([128, PAIRS, S_], F32R)
    v2_sb = projpool.tile([128, HPC_, NKT, HEAD_DIM + 1], F32R)

    with tc.tile_pool(name="xtpool", bufs=1) as xtpool, \
         tc.tile_pool(name="wstage", bufs=1) as wstage, \
         tc.tile_pool(name="wpool", bufs=1) as wpool, \
         tc.tile_pool(name="ph1psum", bufs=2, space="PSUM") as prj_ps, \
         tc.tile_pool(name="vpsum", bufs=2, space="PSUM") as v_ps:
        # x^T arrives as bf16 (host cast); per-nb tiles so the first
        # projection block starts after ~1MB of DMA instead of 4MB
        xTnb = [xtpool.tile([128, CT, PB], BF16, name=f"xT_{nb}")
                for nb in range(NPB)]
        # weights arrive as bf16 from the host: DMA straight in, no rounding
        wq_sb = wpool.tile([128, CT, D2], BF16)
        wk_sb = wpool.tile([128, CT, D2], BF16)
        wv_sb = wpool.tile([128, CT, D2], BF16)
        nc.sync.dma_start(out=wq_sb,
                          in_=wq_d.rearrange("(t p) m -> p t m", p=128))
        xT_r = xT_d.rearrange("(t p) m -> p t m", p=128)
        for nb in range(NPB):
            nc.sync.dma_start(out=xTnb[nb],
                              in_=xT_r[:, :, nb * PB:(nb + 1) * PB])
        for w_sb, w_d in ((wk_sb, wk_d), (wv_sb, wv_d)):
            nc.sync.dma_start(out=w_sb,
                              in_=w_d.rearrange("(t p) m -> p t m", p=128))
        bq_sb = wpool.tile([128, PAIRS, 1], F32)
        bk_sb = wpool.tile([128, PAIRS, 1], F32)
        for b_sb, b_d in ((bq_sb, bq_d), (bk_sb, bk_d)):
            nc.sync.dma_start(
                out=b_sb, in_=b_d.rearrange("(j p) one -> p j one", p=128))
        vbias_sb = wpool.tile([128, D2], F32)
        nc.sync.dma_start(out=vbias_sb, in_=bv_d.to_broadcast([128, D2]))
        for c in range(CT):
            nc.sync.dma_start(out=xh8_sb[:, c, :],
                              in_=xh8_d[c * 128:(c + 1) * 128, :])
        wos = wstage.tile([128, PAIRS, C_], F32, tag="ws", name="wos")
        nc.sync.dma_start(out=wos, in_=wo_d.rearrange("(j p) o -> p j o", p=128))
        nc.vector.tensor_copy(wo_sb, wos)

        for w_sb, b_sb, dest in (
            (wq_sb, bq_sb, qT_sb),
            (wk_sb, bk_sb, kT_sb),
        ):
            for nb in range(NPB):
                pss = [prj_ps.tile([128, PB], F32, tag=f"prj{j}",
                                   name=f"prj_{dest.tensor.name}_{nb}_{j}")
                       for j in range(PAIRS)]
                for c in range(CT):
                    for j in range(PAIRS):
                        nc.tensor.matmul(
                            pss[j],
                            w_sb[:, c, j * 128:(j + 1) * 128],
                            xTnb[nb][:, c, :],
                            start=(c == 0),
                            stop=(c == CT - 1),
                        )
                for j in range(PAIRS):
                    nc.vector.tensor_scalar_add(
                        dest[:, j, nb * PB:(nb + 1) * PB], pss[j], b_sb[:, j, :]
                    )

        # V computed directly with keys on partitions (lhsT = x^T chunk)
        for kt in range(NKT):
            nb, off = divmod(kt, PB // 128)
            off *= 128
            vp = v_ps.tile([128, D2], F32, tag="vp")
            for c in range(CT):
                nc.tensor.matmul(
                    vp,
                    xTnb[nb][:, c, off:off + 128],
                    wv_sb[:, c, :],
                    start=(c == 0),
                    stop=(c == CT - 1),
                )
            nc.vector.tensor_add(
                v2_sb[:, :, kt, 0:HEAD_DIM],
                vp.rearrange("p (h d) -> p h d", h=HPC_),
                vbias_sb.rearrange("p (h d) -> p h d", h=HPC_),
            )
        onescol = wstage.tile([128, HPC_, NKT, 1], F32)
        nc.vector.memset(onescol, WSCALE)
        nc.vector.tensor_copy(v2_sb[:, :, :, HEAD_DIM:HEAD_DIM + 1], onescol)

    # per-qb ctx tiles so the out-projection of early query blocks does not
    # serialize on the last block's divisions (whole-tile dependency)
    ctxT2_nq = [projpool.tile([128, PAIRS, QB], F32R, name=f"ctxT2_{nq}")
                for nq in range(NQB)]

    # --- main loop (phase 2) ---
    ptpool = stack.enter_context(tc.tile_pool(name="ptpool", bufs=13))
    spool = stack.enter_context(tc.tile_pool(name="spool", bufs=2))
    simsb = stack.enter_context(tc.tile_pool(name="simsb", bufs=2))
    smallpool = stack.enter_context(tc.tile_pool(name="smallpool", bufs=2))
    mpool = (stack.enter_context(tc.tile_pool(name="mpool", bufs=2))
             if mask_d is not None else None)

    with tc.tile_pool(name="simpsum", bufs=2, space="PSUM") as simp, \
         tc.tile_pool(name="scpsum", bufs=1, space="PSUM") as scp, \
         tc.tile_pool(name="ctxpsum", bufs=1, space="PSUM") as ctxp:

        def emit_ctx(ctx_ps, kt, pt_pairs):
            for j in range(PAIRS):
                for hi in range(2):
                    nc.tensor.matmul(
                        ctx_ps[2 * j + hi],
                        v2_sb[:, 2 * j + hi, kt, :],
                        pt_pairs[j][:, hi, :],
                        start=(kt == 0),
                        stop=(kt == NKT - 1),
                        skip_group_check=True,
                    )

        def emit_division_head(qb, ctx_ps, h):
            # short chain: single-op approx reciprocal of the denominator
            # row (DVE, PSUM read), GpSimd partition-0 broadcast, DVE mul.
            j, hi = divmod(h, 2)
            r0c = smallpool.tile([1, QB], F32, tag=f"r0c{h % 2}",
                                 name=f"r0c_{qb}_{h}")
            nc.vector.tensor_copy(r0c, ctx_ps[h][HEAD_DIM:HEAD_DIM + 1, :])
            r0 = smallpool.tile([1, QB], F32, tag=f"r0{h % 2}",
                                name=f"r0_{qb}_{h}")
            nc.vector.reciprocal_approx_fast(out=r0, in_=r0c)
            rb = smallpool.tile([HEAD_DIM, QB], F32, tag="rb",
                                name=f"rb_{qb}_{h}")
            nc.gpsimd.partition_broadcast(rb, r0, channels=HEAD_DIM)
            nc.vector.tensor_mul(
                ctxT2_nq[qb][hi * 64:hi * 64 + 64, j, :],
                ctx_ps[h][0:HEAD_DIM, :],
                rb,
            )

        def emit_division(qb, ctx_ps):
            for h in range(HPC_):
                emit_division_head(qb, ctx_ps, h)

        prev_div = None
        for qb in range(NQB):
            ctx_ps = [ctxp.tile([HEAD_DIM + 1, QB], F32, tag=f"ctx{h}",
                                name=f"ctx_{qb}_{h}")
                      for h in range(HPC_)]
            pending = []
            for kt in range(NKT):
                sp = simp.tile([128, QB], F32, tag="sim")
                for cp in range(CP):
                    nc.tensor.matmul(
                        sp,
                        xh8_sb[:, 2 * cp:2 * cp + 2, kt * 128:(kt + 1) * 128],
                        xh8_sb[:, 2 * cp:2 * cp + 2, qb * QB:(qb + 1) * QB],
                        start=(cp == 0),
                        stop=(cp == CP - 1),
                        perf_mode=DOUBLE_ROW,
                    )
                # HW allows only one PSUM operand per DVE op: stage sim in
                # SBUF on ACT (the DVE queue is the oversubscribed one)
                sim_t = simsb.tile([128, QB], F32, tag="simsb")
                nc.scalar.activation(out=sim_t, in_=sp, func=ACT_COPY)
                sim_in = sim_t.unsqueeze(1).to_broadcast([128, 2, QB])
                if mask_d is not None:
                    m_sb = mpool.tile([128, QB], F32, tag="msk")
                    nc.sync.dma_start(
                        out=m_sb,
                        in_=mask_d[kt * 128:(kt + 1) * 128, qb * QB:(qb + 1) * QB],
                    )
                pt_pairs = []
                for j in range(PAIRS):
                    sc_t = scp.tile([128, 2, QB], F32, tag="scp")
                    for hi in range(2):
                        pr = slice(hi * 64, hi * 64 + 64)
                        nc.tensor.matmul(
                            sc_t[:, hi, :],
                            kT_sb[pr, j, kt * 128:(kt + 1) * 128],
                            qT_sb[pr, j, qb * QB:(qb + 1) * QB],
                            start=True,
                            stop=True,
                        )
                    # interleave ctx matmuls (lagged) between the two score
                    # pairs so the PE has work while the DVE runs the sub
                    if j == 0 and pending and len(pending) > LAG:
                        k0, p0 = pending.pop(0)
                        emit_ctx(ctx_ps, k0, p0)
                    s_t = spool.tile([128, 2, QB], F32, tag="s")
                    nc.vector.tensor_sub(s_t, sc_t, sim_in)
                    if mask_d is not None:
                        nc.vector.tensor_sub(
                            s_t, s_t, m_sb.unsqueeze(1).to_broadcast([128, 2, QB]))
                    pt = ptpool.tile([128, 2, QB], F32R, tag="pt")
                    nc.scalar.activation(out=pt, in_=s_t, func=ACT_EXP,
                                         scale=EXP_DESCALE)
                    pt_pairs.append(pt)
                pending.append((kt, pt_pairs))
                # previous block's divisions: one every 2 iterations so all
                # heads are divided by kt=6, when the LAG-5 pipeline emits
                # the first ctx matmuls that need their PSUM banks back
                if prev_div is not None and kt % 2 == 0:
                    h = kt // 2
                    if h < HPC_:
                        emit_division_head(prev_div[0], prev_div[1], h)
                        if h == HPC_ - 1:
                            prev_div = None
            for k0, p0 in pending:
                emit_ctx(ctx_ps, k0, p0)
            if prev_div is not None:
                done = (NKT - 1) // 2 + 1
                for h in range(min(done, HPC_), HPC_):
                    emit_division_head(prev_div[0], prev_div[1], h)
                prev_div = None
            prev_div = (qb, ctx_ps)
        emit_division(*prev_div)

    # --- out-projection (phase 3) ---
    with tc.tile_pool(name="outpsum", bufs=4, space="PSUM") as outp, \
         tc.tile_pool(name="outstg", bufs=4) as outstg:
        QT_PER = QB // 128
        for qt in range(S_ // 128):
            for ob in range(NOB):
                op = outp.tile([128, OB_W], F32, tag="op")
                for j in range(PAIRS):
                    nc.tensor.matmul(
                        op,
                        ctxT2_nq[qt // QT_PER][
                            :, j, (qt % QT_PER) * 128:(qt % QT_PER + 1) * 128],
                        wo_sb[:, j, ob * OB_W:(ob + 1) * OB_W],
                        start=(j == 0),
                        stop=(j == PAIRS - 1),
                    )
                ostg = outstg.tile([128, OB_W], BF16, tag="ostg")
                if (qt + ob) % 2 == 0:
                    nc.scalar.activation(out=ostg, in_=op, func=ACT_COPY)
                else:
                    nc.vector.tensor_copy(ostg, op)
                nc.sync.dma_start(
                    out=out_d[qt * 128:(qt + 1) * 128, ob * OB_W:(ob + 1) * OB_W],
                    in_=ostg,
                )

    stack.close()


def build_nc(*, S_=S, C_=HIDDEN, HPC_=HPC, QB=512, with_mask=False,
             enable_asserts=False):
    nc = bacc.Bacc(
        "TRN2", target_bir_lowering=False, debug=False,
        enable_asserts=enable_asserts,
    )
    D2 = HPC_ * HEAD_DIM
    aps = {}
    aps["xT"] = nc.dram_tensor("xT", [C_, S_], BF16, kind="ExternalInput").ap()
    aps["xh8"] = nc.dram_tensor("xh8", [C_, S_], FP8, kind="ExternalInput").ap()
    for n in ("wq", "wk", "wv"):
        aps[n] = nc.dram_tensor(n, [C_, D2], BF16, kind="ExternalInput").ap()
    aps["wo"] = nc.dram_tensor("wo", [D2, C_], F32, kind="ExternalInput").ap()
    for n in ("bq", "bk"):
        aps[n] = nc.dram_tensor(n, [D2, 1], F32, kind="ExternalInput").ap()
    aps["bv"] = nc.dram_tensor("bv", [1, D2], F32, kind="ExternalInput").ap()
    if with_mask:
        aps["maskadd"] = nc.dram_tensor(
            "maskadd", [S_, S_], F32, kind="ExternalInput").ap()
    aps["out"] = nc.dram_tensor("out", [S_, C_], BF16, kind="ExternalOutput").ap()

    with tile.TileContext(nc) as tc:
        emit_kernel(tc, aps, S_=S_, C_=C_, HPC_=HPC_, QB=QB)
    nc.compile()
    return nc


def _q8(a):
    return a.astype(NP_FP8)


def host_prepare(x, attn_mask, Wq, bq, Wk, bk, Wv, bv, Wo, bo, *,
                 S_=S, C_=HIDDEN, HPC_=HPC, n_cores=N_CORES):
    """Build the per-core input maps. Returns (in_maps, with_mask)."""
    x = np.asarray(x, np.float32)
    B_ = x.shape[0]
    groups = n_cores // B_
    Wq = np.asarray(Wq, np.float32); Wk = np.asarray(Wk, np.float32)
    Wv = np.asarray(Wv, np.float32); Wo = np.asarray(Wo, np.float32)
    bq = np.asarray(bq, np.float32); bk = np.asarray(bk, np.float32)
    bv = np.asarray(bv, np.float32)

    inv_sqrt_d = 1.0 / math.sqrt(HEAD_DIM)
    # weights carry the 256x matched scale so scores_psum = 65536 * s;
    # cast to bf16 on host (device DMAs them straight into SBUF)
    WqT = np.ascontiguousarray(
        (Wq.T * (inv_sqrt_d * WSCALE)).astype(ml_dtypes.bfloat16))
    WkT = np.ascontiguousarray((Wk.T * WSCALE).astype(ml_dtypes.bfloat16))
    WvT = np.ascontiguousarray((Wv.T * WSCALE).astype(ml_dtypes.bfloat16))
    WoT = np.ascontiguousarray(Wo.T)                 # [C(c), C(o)]
    bq_s = bq * (inv_sqrt_d * WSCALE)
    bk_s = bk * WSCALE
    bv_s = bv * WSCALE

    mask = np.asarray(attn_mask)
    with_mask = bool(mask.any())
    maskadd = None
    if with_mask:
        # s_t -= maskadd; masked positions get -1e20/65536 pre-exp -> 0
        maskadd = np.where(mask, np.float32(1e20), np.float32(0.0)).astype(np.float32)
        maskadd = np.ascontiguousarray(maskadd.T)  # [k, q]

    in_maps = []
    per_b = {}
    for core in range(n_cores):
        b, g = divmod(core, groups)
        if b not in per_b:
            xb = x[b]                               # [S, C]
            xT = np.ascontiguousarray(xb.T)         # [C, S]
            norms = np.linalg.norm(xb, axis=1)      # [S]
            scale = (math.sqrt(GAMMA) * WSCALE /
                     np.maximum(norms, 1e-12)).astype(np.float32)
            xh8 = _q8(xT * scale[None, :])
            xT16 = np.ascontiguousarray(xT.astype(ml_dtypes.bfloat16))
            per_b[b] = (xT16, np.ascontiguousarray(xh8))
        xT, xh8 = per_b[b]
        ch = slice(g * HPC_ * HEAD_DIM, (g + 1) * HPC_ * HEAD_DIM)
        m = {
            "xT": xT, "xh8": xh8,
            "wq": np.ascontiguousarray(WqT[:, ch]),
            "wk": np.ascontiguousarray(WkT[:, ch]),
            "wv": np.ascontiguousarray(WvT[:, ch]),
            "wo": np.ascontiguousarray(WoT[ch, :]),
            "bq": np.ascontiguousarray(bq_s[ch]).reshape(-1, 1),
            "bk": np.ascontiguousarray(bk_s[ch]).reshape(-1, 1),
            "bv": np.ascontiguousarray(bv_s[ch]).reshape(1, -1),
        }
        if with_mask:
            m["maskadd"] = maskadd
        in_maps.append(m)
    return in_maps, with_mask


_NC_CACHE = {}


def _get_nc(with_mask):
    key = with_mask
    if key not in _NC_CACHE:
        _NC_CACHE[key] = build_nc(with_mask=with_mask)
    return _NC_CACHE[key]


LAST_RESULTS = None


def kernel(**inputs):
    global LAST_RESULTS
    in_maps, with_mask = host_prepare(
        inputs["x"], inputs["attn_mask"],
        inputs["Wq"], inputs["bq"], inputs["Wk"], inputs["bk"],
        inputs["Wv"], inputs["bv"], inputs["Wo"], inputs["bo"],
    )
    nc = _get_nc(with_mask)
    res = run_bass_kernel_spmd(nc, in_maps, core_ids=list(range(N_CORES)))
    LAST_RESULTS = res
    bo = np.asarray(inputs["bo"], np.float32)
    out = np.zeros((B, S, HIDDEN), np.float32)
    groups = N_CORES // B
    for core in range(N_CORES):
        b = core // groups
        out[b] += res.results[core]["out"].astype(np.float32)
    out += bo[None, None, :]
    return out
